# revision 8
# baseline (speedup 1.0000x reference)
"""Trainium2 Bass kernel for a 3-layer GCN + mean-pool + MLP + softmax.

Reference computation (N=16384 nodes, dense adjacency):
    Ahat = D^-1/2 (A + I) D^-1/2
    H0 = X;  H_{l+1} = relu(Ahat @ (H_l @ W_l) + b_l)   l = 0,1,2
    g = mean(H3, axis=0);  h1 = elu(g @ Wh1 + bh1)
    logits = h1 @ Wh2 + bh2;  probs = softmax(logits)

Distribution (8 NeuronCores, 1D node/row parallel):
  - Host folds the symmetric degree normalization into the adjacency and
    ships each core the *transposed* normalized adjacency columns for its
    2048 output nodes: a_t[k] = Ahat.T[:, k*2048:(k+1)*2048]  (bf16, 64MB).
  - On device, the big matmul per layer streams a_t through the tensor
    engine (moving operand) against the stationary Y_l = H_l @ W_l tiles:
        out.T[c, i] = sum_j Y_l[j, c] * Ahat.T[j, i]   (PSUM fp32 accum)
  - Between layers: each core computes Y_{l+1} rows for its own nodes with
    a small matmul, then an AllGather replicates Y_{l+1} to all cores.
  - Mean pool: per-core partial sum over the free axis + AllReduce, then a
    replicated tiny MLP + softmax; core 0's output is returned.
"""

import numpy as np
import ml_dtypes

N = 16384
NCORES = 8
ROWS = N // NCORES          # 2048 output nodes per core
P = 128
DIMS = [64, 32, 48, 64]     # feature dims: in, after l0, l1, l2
NSTRIPE = 8                 # 128-row j-stripes per DMA group (4MB/group)
NGROUPS = N // (P * NSTRIPE)  # 16
QCH = 512                   # moving-operand free-dim chunk (1 PSUM bank)
NQ = ROWS // QCH            # 4
NU = ROWS // P              # 16 local node tiles
NJT = N // P                # 128 j-tiles per layer

_nc_cache = None


def _build_nc():
    from concourse import bacc, mybir, tile

    dt = mybir.dt
    F32 = dt.float32
    BF16 = dt.bfloat16
    AF = mybir.ActivationFunctionType
    OP = mybir.AluOpType

    nc = bacc.Bacc(
        "TRN2", target_bir_lowering=False, debug=False, num_devices=NCORES
    )

    a_t = nc.dram_tensor("a_t", [N, ROWS], BF16, kind="ExternalInput")
    xt = nc.dram_tensor("xt", [DIMS[0], ROWS], F32, kind="ExternalInput")
    w_d = [
        nc.dram_tensor(f"w{l}", [DIMS[l], DIMS[l + 1]], F32, kind="ExternalInput")
        for l in range(3)
    ]
    b_d = [
        nc.dram_tensor(f"b{l}", [DIMS[l + 1], 1], F32, kind="ExternalInput")
        for l in range(3)
    ]
    wh1_d = nc.dram_tensor("wh1", [DIMS[3], 32], F32, kind="ExternalInput")
    bh1_d = nc.dram_tensor("bh1", [32, 1], F32, kind="ExternalInput")
    wh2_d = nc.dram_tensor("wh2", [32, 2], F32, kind="ExternalInput")
    bh2_d = nc.dram_tensor("bh2", [2, 1], F32, kind="ExternalInput")
    logits_o = nc.dram_tensor("logits", [2, 1], F32, kind="ExternalOutput")
    probs_o = nc.dram_tensor("probs", [2, 1], F32, kind="ExternalOutput")

    rg = [list(range(NCORES))]

    with tile.TileContext(nc) as tc:
        with (
            tc.tile_pool(name="const", bufs=1) as const,
            tc.tile_pool(name="apool", bufs=3) as apool,
            tc.tile_pool(name="spool", bufs=2) as spool,
            tc.tile_pool(name="hpool", bufs=2) as hpool,
            tc.tile_pool(name="ypool", bufs=2) as ypool,
            tc.tile_pool(name="smal", bufs=1) as smal,
            tc.tile_pool(name="accp", bufs=1, space="PSUM") as accp,
            tc.tile_pool(name="psml", bufs=2, space="PSUM") as psml,
            tc.tile_pool(name="psmlp", bufs=1, space="PSUM") as psmlp,
            tc.tile_pool(name="dram", bufs=1, space="DRAM") as dram,
        ):
            # ---- constants into SBUF ----
            def load(handle, shape, name, dtype=F32):
                t = const.tile(shape, dtype, name=name)
                nc.sync.dma_start(t[:], handle.ap())
                return t

            w_sb = [load(w_d[l], [DIMS[l], DIMS[l + 1]], f"w{l}sb") for l in range(3)]
            b_sb = [load(b_d[l], [DIMS[l + 1], 1], f"b{l}sb") for l in range(3)]
            wh1_sb = load(wh1_d, [DIMS[3], 32], "wh1sb")
            bh1_sb = load(bh1_d, [32, 1], "bh1sb")
            wh2_sb = load(wh2_d, [32, 2], "wh2sb")
            bh2_sb = load(bh2_d, [2, 1], "bh2sb")
            xt_sb = load(xt, [DIMS[0], ROWS], "xtsb")

            # ---- Y_l = H_l @ W_l for local nodes, AllGather, reload as
            #      stationary tiles [p, rank, u, c] (node j = r*2048+u*128+p)
            def project_gather(h_sb, l):
                c_out = DIMS[l + 1]
                y_sb = ypool.tile([P, NU, c_out], BF16, tag="y", name=f"y{l}")
                for u in range(NU):
                    ps = psml.tile([P, c_out], F32, tag="psy", name=f"psy{l}_{u}")  # noqa
                    nc.tensor.matmul(
                        ps[:],
                        lhsT=h_sb[:, u * P : (u + 1) * P],
                        rhs=w_sb[l][:],
                        start=True,
                        stop=True,
                    )
                    nc.any.tensor_copy(out=y_sb[:, u, :], in_=ps[:])
                ag_in = dram.tile([P, NU, c_out], BF16, name=f"agin{l}")
                ag_out = dram.tile(
                    [NCORES, P, NU, c_out], BF16, name=f"agout{l}",
                    addr_space="Shared",
                )
                nc.sync.dma_start(ag_in[:], y_sb[:])
                nc.gpsimd.collective_compute(
                    "AllGather",
                    OP.bypass,
                    replica_groups=rg,
                    ins=[ag_in[:].opt()],
                    outs=[ag_out[:].opt()],
                )
                stat = spool.tile(
                    [P, NCORES, NU, c_out], BF16, tag="stat", name=f"stat{l}"
                )
                nc.sync.dma_start(
                    stat[:], ag_out[:].rearrange("r p u c -> p r u c")
                )
                return stat

            stat = project_gather(xt_sb, 0)
            a_re = a_t.ap().rearrange("(g t p) i -> g p t i", t=NSTRIPE, p=P)

            h_sb = None
            for l in range(3):
                c_out = DIMS[l + 1]
                acc = [
                    accp.tile([P, QCH], F32, tag=f"acc{q}", name=f"acc{l}_{q}")
                    for q in range(NQ)
                ]
                for g in range(NGROUPS):
                    a_sb = apool.tile(
                        [P, NSTRIPE, ROWS], BF16, tag="a", name=f"a{l}_{g}"
                    )
                    nc.sync.dma_start(a_sb[:], a_re[g])
                    for t in range(NSTRIPE):
                        jt = g * NSTRIPE + t
                        lw = stat[:, jt // NU, jt % NU, :]
                        for q in range(NQ):
                            nc.tensor.matmul(
                                acc[q][:c_out, :],
                                lhsT=lw,
                                rhs=a_sb[:, t, q * QCH : (q + 1) * QCH],
                                start=(jt == 0),
                                stop=(jt == NJT - 1),
                            )
                h_sb = hpool.tile([c_out, ROWS], F32, tag="h", name=f"h{l}")
                for q in range(NQ):
                    nc.scalar.activation(
                        h_sb[:, q * QCH : (q + 1) * QCH],
                        acc[q][:c_out, :],
                        AF.Relu,
                        bias=b_sb[l][:],
                        scale=1.0,
                    )
                if l < 2:
                    stat = project_gather(h_sb, l + 1)

            # ---- mean pool over all nodes ----
            gp = smal.tile([DIMS[3], 1], F32, name="gpart")
            nc.vector.tensor_reduce(
                gp[:], h_sb[:], axis=mybir.AxisListType.X, op=OP.add
            )
            ar_in = dram.tile([DIMS[3], 1], F32, name="arin")
            ar_out = dram.tile([DIMS[3], 1], F32, name="arout", addr_space="Shared")
            nc.sync.dma_start(ar_in[:], gp[:])
            nc.gpsimd.collective_compute(
                "AllReduce",
                OP.add,
                replica_groups=rg,
                ins=[ar_in[:].opt()],
                outs=[ar_out[:].opt()],
            )
            g_sb = smal.tile([DIMS[3], 1], F32, name="gsb")
            nc.sync.dma_start(g_sb[:], ar_out[:])
            nc.any.tensor_scalar_mul(g_sb[:], g_sb[:], 1.0 / N)

            # ---- MLP head: h1 = elu(g @ Wh1 + bh1) ----
            ps1 = psmlp.tile([32, 1], F32, tag="mlp", name="ps1")
            nc.tensor.matmul(ps1[:], lhsT=wh1_sb[:], rhs=g_sb[:], start=True, stop=True)
            # elu(x) = relu(x) + exp(min(x, 0)) - 1
            tmin = smal.tile([32, 1], F32, name="tmin")
            nc.vector.tensor_scalar(tmin[:], ps1[:], bh1_sb[:], 0.0, OP.add, OP.min)
            e1 = smal.tile([32, 1], F32, name="e1")
            nc.scalar.activation(e1[:], tmin[:], AF.Exp)
            r1 = smal.tile([32, 1], F32, name="r1")
            nc.scalar.activation(r1[:], ps1[:], AF.Relu, bias=bh1_sb[:])
            h1 = smal.tile([32, 1], F32, name="h1")
            nc.vector.tensor_tensor(h1[:], e1[:], r1[:], OP.add)
            nc.vector.tensor_scalar_add(h1[:], h1[:], -1.0)

            # ---- logits = h1 @ Wh2 + bh2; probs = softmax(logits) ----
            ps2 = psmlp.tile([2, 1], F32, tag="mlp", name="ps2")
            nc.tensor.matmul(ps2[:], lhsT=wh2_sb[:], rhs=h1[:], start=True, stop=True)
            logit_sb = smal.tile([2, 1], F32, name="logitsb")
            nc.vector.tensor_scalar(logit_sb[:], ps2[:], bh2_sb[:], None, OP.add)
            nc.sync.dma_start(logits_o.ap(), logit_sb[:])

            e2 = smal.tile([2, 1], F32, name="e2")
            nc.scalar.activation(e2[:], logit_sb[:], AF.Exp)
            ones21 = smal.tile([2, 1], F32, name="ones21")
            nc.any.memset(ones21[:], 1.0)
            ones12 = smal.tile([1, 2], F32, name="ones12")
            nc.any.memset(ones12[:], 1.0)
            ps3 = psmlp.tile([1, 1], F32, tag="mlp", name="ps3")
            nc.tensor.matmul(ps3[:], lhsT=e2[:], rhs=ones21[:], start=True, stop=True)
            rs = smal.tile([1, 1], F32, name="rs")
            nc.vector.reciprocal(rs[:], ps3[:])
            ps4 = psmlp.tile([2, 1], F32, tag="mlp", name="ps4")
            nc.tensor.matmul(ps4[:], lhsT=ones12[:], rhs=rs[:], start=True, stop=True)
            probs_sb = smal.tile([2, 1], F32, name="probssb")
            nc.vector.tensor_tensor(probs_sb[:], e2[:], ps4[:], OP.mult)
            nc.sync.dma_start(probs_o.ap(), probs_sb[:])

    nc.finalize()
    return nc


def _install_ntff_hook():
    """Register the axon NTFF profiling hook if the container's antenv stub
    lacks it (bass_utils imports antenv.axon_hooks when trace=True)."""
    import sys
    import types

    try:
        import antenv.axon_hooks  # noqa: F401
        return
    except ImportError:
        pass
    mod = types.ModuleType("antenv.axon_hooks")
    _h = [None]
    mod.set_axon_ntff_profile_hook = lambda h: _h.__setitem__(0, h)
    mod.get_axon_ntff_profile_hook = lambda: _h[0]
    sys.modules["antenv.axon_hooks"] = mod
    import antenv

    antenv.axon_hooks = mod
    try:
        from trn_agent_boot import trn_boot

        hook = trn_boot._ntff_profile_via_ctypes("/opt/axon/libaxon_pjrt.so")
        if hook is not None:
            mod.set_axon_ntff_profile_hook(hook)
    except Exception:
        pass


def _get_nc():
    global _nc_cache
    if _nc_cache is None:
        _nc_cache = _build_nc()
    return _nc_cache


_last_results = None


def kernel(
    node_feat,
    adj_matrix,
    W0,
    b0,
    W1,
    b1,
    W2,
    b2,
    Wh1,
    bh1,
    Wh2,
    bh2,
):
    global _last_results
    import os

    node_feat = np.ascontiguousarray(np.asarray(node_feat, dtype=np.float32))
    adj = np.asarray(adj_matrix, dtype=np.float32)

    # ---- host-side sharding / preprocessing ----
    deg = adj.sum(axis=1, dtype=np.float32) + 1.0
    dinv = (1.0 / np.sqrt(deg)).astype(np.float32)

    bf16 = ml_dtypes.bfloat16
    f32c = lambda a, shape=None: np.ascontiguousarray(
        np.asarray(a, dtype=np.float32).reshape(shape)
        if shape is not None
        else np.asarray(a, dtype=np.float32)
    )

    common = {
        "w0": f32c(W0),
        "b0": f32c(b0, (-1, 1)),
        "w1": f32c(W1),
        "b1": f32c(b1, (-1, 1)),
        "w2": f32c(W2),
        "b2": f32c(b2, (-1, 1)),
        "wh1": f32c(Wh1),
        "bh1": f32c(bh1, (-1, 1)),
        "wh2": f32c(Wh2),
        "bh2": f32c(bh2, (-1, 1)),
    }

    in_maps = []
    idx = np.arange(ROWS)
    for k in range(NCORES):
        sl = slice(k * ROWS, (k + 1) * ROWS)
        # rows of Ahat for this core's output nodes, from raw adjacency rows
        blk = adj[sl, :] * dinv[sl, None]
        blk *= dinv[None, :]
        blk[idx, k * ROWS + idx] = dinv[sl] * dinv[sl]  # + I self loops
        a_k = blk.T.astype(bf16)  # [N, ROWS] = Ahat.T columns, C-contiguous
        xt_k = np.ascontiguousarray(node_feat[sl, :].T)  # [64, ROWS]
        m = {"a_t": a_k, "xt": xt_k}
        m.update(common)
        in_maps.append(m)

    from concourse import bass_utils

    nc = _get_nc()
    trace = bool(int(os.environ.get("GCN_TRACE", "0")))
    if trace:
        _install_ntff_hook()
    res = bass_utils.run_bass_kernel_spmd(
        nc, in_maps, core_ids=list(range(NCORES)), trace=trace
    )
    _last_results = res

    out0 = res.results[0]
    logits = np.asarray(out0["logits"], dtype=np.float32).reshape(2)
    probs = np.asarray(out0["probs"], dtype=np.float32).reshape(2)
    return (logits, probs)


# revision 9
# speedup vs baseline: 1.5098x; 1.5098x over previous
"""Trainium2 Bass kernel for a 3-layer GCN + mean-pool + MLP + softmax.

Reference computation (N=16384 nodes, dense adjacency):
    Ahat = D^-1/2 (A + I) D^-1/2
    H0 = X;  H_{l+1} = relu(Ahat @ (H_l @ W_l) + b_l)   l = 0,1,2
    g = mean(H3, axis=0);  h1 = elu(g @ Wh1 + bh1)
    logits = h1 @ Wh2 + bh2;  probs = softmax(logits)

Distribution (8 NeuronCores, 1D node/row parallel):
  - Host folds the symmetric degree normalization into the adjacency and
    ships each core the *transposed* normalized adjacency columns for its
    2048 output nodes: a_t[k] = (ASCALE * Ahat.T)[:, k*2048:(k+1)*2048]
    as fp8 e4m3 (32MB/core).  ASCALE=16 keeps entries in fp8 normal range;
    it is divided back out by the relu activation's scale parameter.
  - On device, the big matmul per layer streams a_t through the tensor
    engine (moving operand, DoubleRow fp8: 256-deep contraction) against
    the stationary Y_l = H_l @ W_l tiles:
        out.T[c, i] = sum_j Y_l[j, c] * Ahat.T[j, i]   (PSUM fp32 accum)
  - Between layers: each core computes Y_{l+1} rows for its own nodes with
    a small fp32 matmul, then an AllGather replicates Y_{l+1} to all cores.
  - Mean pool: per-core partial sum over the free axis + AllReduce, then a
    replicated tiny MLP + softmax; core 0's output is returned.
  - DMA ring split: the bulk adjacency stream runs on the SP (nc.sync)
    HWDGE ring; all small loads that must wait on collectives run on the
    ACT (nc.scalar) ring so they never stall the adjacency stream.
"""

import numpy as np
import ml_dtypes

N = 16384
NCORES = 8
ROWS = N // NCORES          # 2048 output nodes per core
P = 128
DIMS = [64, 32, 48, 64]     # feature dims: in, after l0, l1, l2
NSTRIPE = 16                # 128-row j-stripes per DMA group (4MB fp8)
NGROUPS = N // (P * NSTRIPE)  # 8
QCH = 512                   # moving-operand free-dim chunk (1 PSUM bank)
NQ = ROWS // QCH            # 4
NU = ROWS // P              # 16 local node tiles
NDT = N // (2 * P)          # 64 double j-tiles per layer (DoubleRow)
ASCALE = 16.0               # fp8 range helper, divided out in the relu

_nc_cache = None


def _build_nc():
    from concourse import bacc, mybir, tile

    dt = mybir.dt
    F32 = dt.float32
    F8 = dt.float8e4
    AF = mybir.ActivationFunctionType
    OP = mybir.AluOpType
    DR = mybir.MatmulPerfMode.DoubleRow

    nc = bacc.Bacc(
        "TRN2", target_bir_lowering=False, debug=False, num_devices=NCORES
    )

    a_t = nc.dram_tensor("a_t", [N, ROWS], F8, kind="ExternalInput")
    xt = nc.dram_tensor("xt", [DIMS[0], ROWS], F32, kind="ExternalInput")
    w_d = [
        nc.dram_tensor(f"w{l}", [DIMS[l], DIMS[l + 1]], F32, kind="ExternalInput")
        for l in range(3)
    ]
    b_d = [
        nc.dram_tensor(f"b{l}", [DIMS[l + 1], 1], F32, kind="ExternalInput")
        for l in range(3)
    ]
    wh1_d = nc.dram_tensor("wh1", [DIMS[3], 32], F32, kind="ExternalInput")
    bh1_d = nc.dram_tensor("bh1", [32, 1], F32, kind="ExternalInput")
    wh2_d = nc.dram_tensor("wh2", [32, 2], F32, kind="ExternalInput")
    bh2_d = nc.dram_tensor("bh2", [2, 1], F32, kind="ExternalInput")
    logits_o = nc.dram_tensor("logits", [2, 1], F32, kind="ExternalOutput")
    probs_o = nc.dram_tensor("probs", [2, 1], F32, kind="ExternalOutput")

    rg = [list(range(NCORES))]

    with tile.TileContext(nc) as tc:
        with (
            tc.tile_pool(name="const", bufs=1) as const,
            tc.tile_pool(name="apool", bufs=4) as apool,
            tc.tile_pool(name="spool", bufs=2) as spool,
            tc.tile_pool(name="hpool", bufs=2) as hpool,
            tc.tile_pool(name="ypool", bufs=2) as ypool,
            tc.tile_pool(name="smal", bufs=1) as smal,
            tc.tile_pool(name="accp", bufs=1, space="PSUM") as accp,
            tc.tile_pool(name="psml", bufs=2, space="PSUM") as psml,
            tc.tile_pool(name="psmlp", bufs=1, space="PSUM") as psmlp,
            tc.tile_pool(name="dram", bufs=1, space="DRAM") as dram,
        ):
            # ---- constants into SBUF (ACT ring — keep SP ring for A) ----
            def load(handle, shape, name, dtype=F32):
                t = const.tile(shape, dtype, name=name)
                nc.scalar.dma_start(t[:], handle.ap())
                return t

            w_sb = [load(w_d[l], [DIMS[l], DIMS[l + 1]], f"w{l}sb") for l in range(3)]
            b_sb = [load(b_d[l], [DIMS[l + 1], 1], f"b{l}sb") for l in range(3)]
            wh1_sb = load(wh1_d, [DIMS[3], 32], "wh1sb")
            bh1_sb = load(bh1_d, [32, 1], "bh1sb")
            wh2_sb = load(wh2_d, [32, 2], "wh2sb")
            bh2_sb = load(bh2_d, [2, 1], "bh2sb")
            xt_sb = load(xt, [DIMS[0], ROWS], "xtsb")

            # ---- Y_l = H_l @ W_l for local nodes, AllGather, reload as
            #      stationary tiles [p, rank, u, c] (node j = r*2048+u*128+p)
            def project_gather(h_sb, l):
                c_out = DIMS[l + 1]
                y_sb = ypool.tile([P, NU, c_out], F8, tag="y", name=f"y{l}")
                for u in range(NU):
                    ps = psml.tile([P, c_out], F32, tag="psy", name=f"psy{l}_{u}")
                    nc.tensor.matmul(
                        ps[:],
                        lhsT=h_sb[:, u * P : (u + 1) * P],
                        rhs=w_sb[l][:],
                        start=True,
                        stop=True,
                    )
                    nc.any.tensor_copy(out=y_sb[:, u, :], in_=ps[:])
                ag_in = dram.tile([P, NU, c_out], F8, name=f"agin{l}")
                ag_out = dram.tile(
                    [NCORES, P, NU, c_out], F8, name=f"agout{l}",
                    addr_space="Shared",
                )
                nc.scalar.dma_start(ag_in[:], y_sb[:])
                nc.gpsimd.collective_compute(
                    "AllGather",
                    OP.bypass,
                    replica_groups=rg,
                    ins=[ag_in[:].opt()],
                    outs=[ag_out[:].opt()],
                )
                stat = spool.tile(
                    [P, NCORES, NU, c_out], F8, tag="stat", name=f"stat{l}"
                )
                nc.scalar.dma_start(
                    stat[:], ag_out[:].rearrange("r p u c -> p r u c")
                )
                return stat

            stat = project_gather(xt_sb, 0)
            a_re = a_t.ap().rearrange("(g t p) i -> g p t i", t=NSTRIPE, p=P)

            h_sb = None
            for l in range(3):
                c_out = DIMS[l + 1]
                acc = [
                    accp.tile([P, QCH], F32, tag=f"acc{q}", name=f"acc{l}_{q}")
                    for q in range(NQ)
                ]
                for g in range(NGROUPS):
                    a_sb = apool.tile(
                        [P, NSTRIPE, ROWS], F8, tag="a", name=f"a{l}_{g}"
                    )
                    nc.sync.dma_start(a_sb[:], a_re[g])
                    for t in range(NSTRIPE // 2):
                        jd = g * (NSTRIPE // 2) + t       # double j-tile index
                        jt = 2 * jd
                        lw = stat[:, jt // NU, (jt % NU) : (jt % NU) + 2, :]
                        for q in range(NQ):
                            nc.tensor.matmul(
                                acc[q][:c_out, :],
                                lhsT=lw,
                                rhs=a_sb[:, 2 * t : 2 * t + 2, q * QCH : (q + 1) * QCH],
                                start=(jd == 0),
                                stop=(jd == NDT - 1),
                                perf_mode=DR,
                            )
                h_sb = hpool.tile([c_out, ROWS], F32, tag="h", name=f"h{l}")
                for q in range(NQ):
                    nc.scalar.activation(
                        h_sb[:, q * QCH : (q + 1) * QCH],
                        acc[q][:c_out, :],
                        AF.Relu,
                        bias=b_sb[l][:],
                        scale=1.0 / ASCALE,
                    )
                if l < 2:
                    stat = project_gather(h_sb, l + 1)

            # ---- mean pool over all nodes ----
            gp = smal.tile([DIMS[3], 1], F32, name="gpart")
            nc.vector.tensor_reduce(
                gp[:], h_sb[:], axis=mybir.AxisListType.X, op=OP.add
            )
            ar_in = dram.tile([DIMS[3], 1], F32, name="arin")
            ar_out = dram.tile([DIMS[3], 1], F32, name="arout", addr_space="Shared")
            nc.scalar.dma_start(ar_in[:], gp[:])
            nc.gpsimd.collective_compute(
                "AllReduce",
                OP.add,
                replica_groups=rg,
                ins=[ar_in[:].opt()],
                outs=[ar_out[:].opt()],
            )
            g_sb = smal.tile([DIMS[3], 1], F32, name="gsb")
            nc.scalar.dma_start(g_sb[:], ar_out[:])
            nc.any.tensor_scalar_mul(g_sb[:], g_sb[:], 1.0 / N)

            # ---- MLP head: h1 = elu(g @ Wh1 + bh1) ----
            ps1 = psmlp.tile([32, 1], F32, tag="mlp", name="ps1")
            nc.tensor.matmul(ps1[:], lhsT=wh1_sb[:], rhs=g_sb[:], start=True, stop=True)
            # elu(x) = relu(x) + exp(min(x, 0)) - 1
            tmin = smal.tile([32, 1], F32, name="tmin")
            nc.vector.tensor_scalar(tmin[:], ps1[:], bh1_sb[:], 0.0, OP.add, OP.min)
            e1 = smal.tile([32, 1], F32, name="e1")
            nc.scalar.activation(e1[:], tmin[:], AF.Exp)
            r1 = smal.tile([32, 1], F32, name="r1")
            nc.scalar.activation(r1[:], ps1[:], AF.Relu, bias=bh1_sb[:])
            h1 = smal.tile([32, 1], F32, name="h1")
            nc.vector.tensor_tensor(h1[:], e1[:], r1[:], OP.add)
            nc.vector.tensor_scalar_add(h1[:], h1[:], -1.0)

            # ---- logits = h1 @ Wh2 + bh2; probs = softmax(logits) ----
            ps2 = psmlp.tile([2, 1], F32, tag="mlp", name="ps2")
            nc.tensor.matmul(ps2[:], lhsT=wh2_sb[:], rhs=h1[:], start=True, stop=True)
            logit_sb = smal.tile([2, 1], F32, name="logitsb")
            nc.vector.tensor_scalar(logit_sb[:], ps2[:], bh2_sb[:], None, OP.add)
            nc.scalar.dma_start(logits_o.ap(), logit_sb[:])

            e2 = smal.tile([2, 1], F32, name="e2")
            nc.scalar.activation(e2[:], logit_sb[:], AF.Exp)
            ones21 = smal.tile([2, 1], F32, name="ones21")
            nc.any.memset(ones21[:], 1.0)
            ones12 = smal.tile([1, 2], F32, name="ones12")
            nc.any.memset(ones12[:], 1.0)
            ps3 = psmlp.tile([1, 1], F32, tag="mlp", name="ps3")
            nc.tensor.matmul(ps3[:], lhsT=e2[:], rhs=ones21[:], start=True, stop=True)
            rs = smal.tile([1, 1], F32, name="rs")
            nc.vector.reciprocal(rs[:], ps3[:])
            ps4 = psmlp.tile([2, 1], F32, tag="mlp", name="ps4")
            nc.tensor.matmul(ps4[:], lhsT=ones12[:], rhs=rs[:], start=True, stop=True)
            probs_sb = smal.tile([2, 1], F32, name="probssb")
            nc.vector.tensor_tensor(probs_sb[:], e2[:], ps4[:], OP.mult)
            nc.scalar.dma_start(probs_o.ap(), probs_sb[:])

    nc.finalize()
    return nc


def _install_ntff_hook():
    """Register the axon NTFF profiling hook if the container's antenv stub
    lacks it (bass_utils imports antenv.axon_hooks when trace=True)."""
    import sys
    import types

    try:
        import antenv.axon_hooks  # noqa: F401
        return
    except ImportError:
        pass
    mod = types.ModuleType("antenv.axon_hooks")
    _h = [None]
    mod.set_axon_ntff_profile_hook = lambda h: _h.__setitem__(0, h)
    mod.get_axon_ntff_profile_hook = lambda: _h[0]
    sys.modules["antenv.axon_hooks"] = mod
    import antenv

    antenv.axon_hooks = mod
    try:
        from trn_agent_boot import trn_boot

        hook = trn_boot._ntff_profile_via_ctypes("/opt/axon/libaxon_pjrt.so")
        if hook is not None:
            mod.set_axon_ntff_profile_hook(hook)
    except Exception:
        pass


def _get_nc():
    global _nc_cache
    if _nc_cache is None:
        _nc_cache = _build_nc()
    return _nc_cache


_last_results = None


def kernel(
    node_feat,
    adj_matrix,
    W0,
    b0,
    W1,
    b1,
    W2,
    b2,
    Wh1,
    bh1,
    Wh2,
    bh2,
):
    global _last_results
    import os

    node_feat = np.ascontiguousarray(np.asarray(node_feat, dtype=np.float32))
    adj = np.asarray(adj_matrix, dtype=np.float32)

    # ---- host-side sharding / preprocessing ----
    deg = adj.sum(axis=1, dtype=np.float32) + 1.0
    dinv = (1.0 / np.sqrt(deg)).astype(np.float32)

    fp8 = ml_dtypes.float8_e4m3
    f32c = lambda a, shape=None: np.ascontiguousarray(
        np.asarray(a, dtype=np.float32).reshape(shape)
        if shape is not None
        else np.asarray(a, dtype=np.float32)
    )

    common = {
        "w0": f32c(W0),
        "b0": f32c(b0, (-1, 1)),
        "w1": f32c(W1),
        "b1": f32c(b1, (-1, 1)),
        "w2": f32c(W2),
        "b2": f32c(b2, (-1, 1)),
        "wh1": f32c(Wh1),
        "bh1": f32c(bh1, (-1, 1)),
        "wh2": f32c(Wh2),
        "bh2": f32c(bh2, (-1, 1)),
    }

    in_maps = []
    idx = np.arange(ROWS)
    sdinv = dinv * np.float32(ASCALE)
    for k in range(NCORES):
        sl = slice(k * ROWS, (k + 1) * ROWS)
        # rows of ASCALE*Ahat for this core's output nodes
        blk = adj[sl, :] * sdinv[sl, None]
        blk *= dinv[None, :]
        blk[idx, k * ROWS + idx] = sdinv[sl] * dinv[sl]  # + I self loops
        a_k = blk.T.astype(fp8)  # [N, ROWS] = scaled Ahat.T cols, contiguous
        xt_k = np.ascontiguousarray(node_feat[sl, :].T)  # [64, ROWS]
        m = {"a_t": a_k, "xt": xt_k}
        m.update(common)
        in_maps.append(m)

    from concourse import bass_utils

    nc = _get_nc()
    trace = bool(int(os.environ.get("GCN_TRACE", "0")))
    if trace:
        _install_ntff_hook()
    res = bass_utils.run_bass_kernel_spmd(
        nc, in_maps, core_ids=list(range(NCORES)), trace=trace
    )
    _last_results = res

    out0 = res.results[0]
    logits = np.asarray(out0["logits"], dtype=np.float32).reshape(2)
    probs = np.asarray(out0["probs"], dtype=np.float32).reshape(2)
    return (logits, probs)


# revision 15
# speedup vs baseline: 1.7817x; 1.1801x over previous
"""Trainium2 Bass kernel for a 3-layer GCN + mean-pool + MLP + softmax.

Reference computation (N=16384 nodes, dense adjacency):
    Ahat = D^-1/2 (A + I) D^-1/2
    H0 = X;  H_{l+1} = relu(Ahat @ (H_l @ W_l) + b_l)   l = 0,1,2
    g = mean(H3, axis=0);  h1 = elu(g @ Wh1 + bh1)
    logits = h1 @ Wh2 + bh2;  probs = softmax(logits)

Distribution (8 NeuronCores, 1D node/row parallel):
  - Host folds the symmetric degree normalization into the adjacency and
    ships each core the *transposed* normalized adjacency columns for its
    2048 output nodes: a_t[k] = (ASCALE * Ahat.T)[:, k*2048:(k+1)*2048]
    as fp8 e4m3 (32MB/core).  ASCALE=16 keeps entries in fp8 normal range;
    it is divided back out by the relu activation's scale parameter.
  - On device, the big matmul per layer streams a_t through the tensor
    engine (moving operand, DoubleRow fp8: 256-deep contraction) against
    the stationary Y_l = H_l @ W_l tiles:
        out.T[c, i] = sum_j Y_l[j, c] * Ahat.T[j, i]   (PSUM fp32 accum)
  - Between layers: each core computes Y_{l+1} rows for its own nodes with
    a small fp32 matmul, then an AllGather replicates Y_{l+1} to all cores.
  - Mean pool: per-core partial sum over the free axis + AllReduce, then a
    replicated tiny MLP + softmax; core 0's output is returned.
  - DMA ring split: the bulk adjacency stream runs on the SP (nc.sync)
    HWDGE ring; all small loads that must wait on collectives run on the
    ACT (nc.scalar) ring so they never stall the adjacency stream.
"""

import numpy as np
import ml_dtypes

N = 16384
NCORES = 8
ROWS = N // NCORES          # 2048 output nodes per core
P = 128
DIMS = [64, 32, 48, 64]     # feature dims: in, after l0, l1, l2
NSTRIPE = 16                # 128-row j-stripes per DMA group (4MB fp8)
NGROUPS = N // (P * NSTRIPE)  # 8
QCH = 512                   # moving-operand free-dim chunk (1 PSUM bank)
NQ = ROWS // QCH            # 4
NU = ROWS // P              # 16 local node tiles
NDT = N // (2 * P)          # 64 double j-tiles per layer (DoubleRow)
ASCALE = 16.0               # fp8 range helper, divided out in the relu

_nc_cache = None


def _build_nc():
    from concourse import bacc, mybir, tile

    dt = mybir.dt
    F32 = dt.float32
    F8 = dt.float8e4
    AF = mybir.ActivationFunctionType
    OP = mybir.AluOpType
    DR = mybir.MatmulPerfMode.DoubleRow

    nc = bacc.Bacc(
        "TRN2", target_bir_lowering=False, debug=False, num_devices=NCORES
    )

    BF16 = dt.bfloat16
    a_t = nc.dram_tensor("a_t", [N, ROWS], F8, kind="ExternalInput")
    # full X.T, replicated: every core computes the whole Y0 = X @ W0 locally
    # (no AllGather before layer 1, so the collective entry barrier is hidden)
    xt = nc.dram_tensor("xt", [DIMS[0], N], BF16, kind="ExternalInput")
    w_d = [
        nc.dram_tensor(
            f"w{l}", [DIMS[l], DIMS[l + 1]], BF16 if l == 0 else F32,
            kind="ExternalInput",
        )
        for l in range(3)
    ]
    b_d = [
        nc.dram_tensor(f"b{l}", [DIMS[l + 1], 1], F32, kind="ExternalInput")
        for l in range(3)
    ]
    wh1_d = nc.dram_tensor("wh1", [DIMS[3], 32], F32, kind="ExternalInput")
    bh1_d = nc.dram_tensor("bh1", [32, 1], F32, kind="ExternalInput")
    wh2_d = nc.dram_tensor("wh2", [32, 2], F32, kind="ExternalInput")
    bh2_d = nc.dram_tensor("bh2", [2, 1], F32, kind="ExternalInput")
    logits_o = nc.dram_tensor("logits", [2, 1], F32, kind="ExternalOutput")
    probs_o = nc.dram_tensor("probs", [2, 1], F32, kind="ExternalOutput")

    rg = [list(range(NCORES))]

    with tile.TileContext(nc) as tc:
        with (
            tc.tile_pool(name="const", bufs=1) as const,
            tc.tile_pool(name="apool", bufs=4) as apool,
            tc.tile_pool(name="spool", bufs=2) as spool,
            tc.tile_pool(name="hpool", bufs=2) as hpool,
            tc.tile_pool(name="ypool", bufs=2) as ypool,
            tc.tile_pool(name="smal", bufs=1) as smal,
            tc.tile_pool(name="accp", bufs=1, space="PSUM") as accp,
            tc.tile_pool(name="psml", bufs=2, space="PSUM") as psml,
            tc.tile_pool(name="psmlp", bufs=1, space="PSUM") as psmlp,
            tc.tile_pool(name="dram", bufs=1, space="DRAM") as dram,
        ):
            # ---- constants into SBUF (ACT ring — keep SP ring for A) ----
            def load(handle, shape, name, dtype=F32):
                t = const.tile(shape, dtype, name=name)
                nc.scalar.dma_start(t[:], handle.ap())
                return t

            w_sb = [
                load(
                    w_d[l], [DIMS[l], DIMS[l + 1]], f"w{l}sb",
                    dtype=BF16 if l == 0 else F32,
                )
                for l in range(3)
            ]
            b_sb = [load(b_d[l], [DIMS[l + 1], 1], f"b{l}sb") for l in range(3)]
            wh1_sb = load(wh1_d, [DIMS[3], 32], "wh1sb")
            bh1_sb = load(bh1_d, [32, 1], "bh1sb")
            wh2_sb = load(wh2_d, [32, 2], "wh2sb")
            bh2_sb = load(bh2_d, [2, 1], "bh2sb")
            xt_sb = load(xt, [DIMS[0], N], "xtsb", dtype=BF16)

            # ---- Y_l = H_l @ W_l for local nodes, AllGather, reload as
            #      stationary tiles [p, rank, u, c] (node j = r*2048+u*128+p)
            def project_gather(h_sb, l):
                c_out = DIMS[l + 1]
                y_sb = ypool.tile([P, NU, c_out], F8, tag="y", name=f"y{l}")
                for u in range(NU):
                    ps = psml.tile([P, c_out], F32, tag="psy", name=f"psy{l}_{u}")
                    nc.tensor.matmul(
                        ps[:],
                        lhsT=h_sb[:, u * P : (u + 1) * P],
                        rhs=w_sb[l][:],
                        start=True,
                        stop=True,
                    )
                    nc.any.tensor_copy(out=y_sb[:, u, :], in_=ps[:])
                ag_in = dram.tile([P, NU, c_out], F8, name=f"agin{l}")
                ag_out = dram.tile(
                    [NCORES, P, NU, c_out], F8, name=f"agout{l}",
                    addr_space="Shared",
                )
                nc.scalar.dma_start(ag_in[:], y_sb[:])
                nc.gpsimd.collective_compute(
                    "AllGather",
                    OP.bypass,
                    replica_groups=rg,
                    ins=[ag_in[:].opt()],
                    outs=[ag_out[:].opt()],
                )
                stat = spool.tile(
                    [P, NCORES, NU, c_out], F8, tag="stat", name=f"stat{l}"
                )
                nc.scalar.dma_start(
                    stat[:], ag_out[:].rearrange("r p u c -> p r u c")
                )
                return stat

            # ---- layer-0 stationary: full Y0 = X @ W0, computed locally ----
            stat = spool.tile(
                [P, NCORES, NU, DIMS[1]], F8, tag="stat", name="stat0"
            )
            for jt in range(N // P):
                ps = psml.tile([P, DIMS[1]], F32, tag="psy", name=f"psy0_{jt}")
                nc.tensor.matmul(
                    ps[:],
                    lhsT=xt_sb[:, jt * P : (jt + 1) * P],
                    rhs=w_sb[0][:],
                    start=True,
                    stop=True,
                )
                nc.any.tensor_copy(out=stat[:, jt // NU, jt % NU, :], in_=ps[:])

            a_re = a_t.ap().rearrange("(g t p) i -> g p t i", t=NSTRIPE, p=P)

            h_sb = None
            for l in range(3):
                c_out = DIMS[l + 1]
                acc = [
                    accp.tile([P, QCH], F32, tag=f"acc{q}", name=f"acc{l}_{q}")
                    for q in range(NQ)
                ]
                for g in range(NGROUPS):
                    a_sb = apool.tile(
                        [P, NSTRIPE, ROWS], F8, tag="a", name=f"a{l}_{g}"
                    )
                    nc.sync.dma_start(a_sb[:], a_re[g])
                    for t in range(NSTRIPE // 2):
                        jd = g * (NSTRIPE // 2) + t       # double j-tile index
                        jt = 2 * jd
                        lw = stat[:, jt // NU, (jt % NU) : (jt % NU) + 2, :]
                        for q in range(NQ):
                            nc.tensor.matmul(
                                acc[q][:c_out, :],
                                lhsT=lw,
                                rhs=a_sb[:, 2 * t : 2 * t + 2, q * QCH : (q + 1) * QCH],
                                start=(jd == 0),
                                stop=(jd == NDT - 1),
                                perf_mode=DR,
                            )
                h_sb = hpool.tile([c_out, ROWS], F32, tag="h", name=f"h{l}")
                for q in range(NQ):
                    nc.scalar.activation(
                        h_sb[:, q * QCH : (q + 1) * QCH],
                        acc[q][:c_out, :],
                        AF.Relu,
                        bias=b_sb[l][:],
                        scale=1.0 / ASCALE,
                    )
                if l < 2:
                    stat = project_gather(h_sb, l + 1)

            # ---- mean pool over all nodes ----
            gp = smal.tile([DIMS[3], 1], F32, name="gpart")
            nc.vector.tensor_reduce(
                gp[:], h_sb[:], axis=mybir.AxisListType.X, op=OP.add
            )
            ar_in = dram.tile([DIMS[3], 1], F32, name="arin")
            ar_out = dram.tile([DIMS[3], 1], F32, name="arout", addr_space="Shared")
            nc.scalar.dma_start(ar_in[:], gp[:])
            nc.gpsimd.collective_compute(
                "AllReduce",
                OP.add,
                replica_groups=rg,
                ins=[ar_in[:].opt()],
                outs=[ar_out[:].opt()],
            )
            g_sb = smal.tile([DIMS[3], 1], F32, name="gsb")
            nc.scalar.dma_start(g_sb[:], ar_out[:])
            nc.any.tensor_scalar_mul(g_sb[:], g_sb[:], 1.0 / N)

            # ---- MLP head: h1 = elu(g @ Wh1 + bh1) ----
            ps1 = psmlp.tile([32, 1], F32, tag="mlp", name="ps1")
            nc.tensor.matmul(ps1[:], lhsT=wh1_sb[:], rhs=g_sb[:], start=True, stop=True)
            # elu(x) = relu(x) + exp(min(x, 0)) - 1
            tmin = smal.tile([32, 1], F32, name="tmin")
            nc.vector.tensor_scalar(tmin[:], ps1[:], bh1_sb[:], 0.0, OP.add, OP.min)
            e1 = smal.tile([32, 1], F32, name="e1")
            nc.scalar.activation(e1[:], tmin[:], AF.Exp)
            r1 = smal.tile([32, 1], F32, name="r1")
            nc.scalar.activation(r1[:], ps1[:], AF.Relu, bias=bh1_sb[:])
            h1 = smal.tile([32, 1], F32, name="h1")
            nc.vector.tensor_tensor(h1[:], e1[:], r1[:], OP.add)
            nc.vector.tensor_scalar_add(h1[:], h1[:], -1.0)

            # ---- logits = h1 @ Wh2 + bh2; probs = softmax(logits) ----
            ps2 = psmlp.tile([2, 1], F32, tag="mlp", name="ps2")
            nc.tensor.matmul(ps2[:], lhsT=wh2_sb[:], rhs=h1[:], start=True, stop=True)
            logit_sb = smal.tile([2, 1], F32, name="logitsb")
            nc.vector.tensor_scalar(logit_sb[:], ps2[:], bh2_sb[:], None, OP.add)
            nc.scalar.dma_start(logits_o.ap(), logit_sb[:])

            e2 = smal.tile([2, 1], F32, name="e2")
            nc.scalar.activation(e2[:], logit_sb[:], AF.Exp)
            ones21 = smal.tile([2, 1], F32, name="ones21")
            nc.any.memset(ones21[:], 1.0)
            ones12 = smal.tile([1, 2], F32, name="ones12")
            nc.any.memset(ones12[:], 1.0)
            ps3 = psmlp.tile([1, 1], F32, tag="mlp", name="ps3")
            nc.tensor.matmul(ps3[:], lhsT=e2[:], rhs=ones21[:], start=True, stop=True)
            rs = smal.tile([1, 1], F32, name="rs")
            nc.vector.reciprocal(rs[:], ps3[:])
            ps4 = psmlp.tile([2, 1], F32, tag="mlp", name="ps4")
            nc.tensor.matmul(ps4[:], lhsT=ones12[:], rhs=rs[:], start=True, stop=True)
            probs_sb = smal.tile([2, 1], F32, name="probssb")
            nc.vector.tensor_tensor(probs_sb[:], e2[:], ps4[:], OP.mult)
            nc.scalar.dma_start(probs_o.ap(), probs_sb[:])

    nc.finalize()
    return nc


def _install_ntff_hook():
    """Register the axon NTFF profiling hook if the container's antenv stub
    lacks it (bass_utils imports antenv.axon_hooks when trace=True)."""
    import sys
    import types

    try:
        import antenv.axon_hooks  # noqa: F401
        return
    except ImportError:
        pass
    mod = types.ModuleType("antenv.axon_hooks")
    _h = [None]
    mod.set_axon_ntff_profile_hook = lambda h: _h.__setitem__(0, h)
    mod.get_axon_ntff_profile_hook = lambda: _h[0]
    sys.modules["antenv.axon_hooks"] = mod
    import antenv

    antenv.axon_hooks = mod
    try:
        from trn_agent_boot import trn_boot

        hook = trn_boot._ntff_profile_via_ctypes("/opt/axon/libaxon_pjrt.so")
        if hook is not None:
            mod.set_axon_ntff_profile_hook(hook)
    except Exception:
        pass


def _get_nc():
    global _nc_cache
    if _nc_cache is None:
        _nc_cache = _build_nc()
    return _nc_cache


_last_results = None


def kernel(
    node_feat,
    adj_matrix,
    W0,
    b0,
    W1,
    b1,
    W2,
    b2,
    Wh1,
    bh1,
    Wh2,
    bh2,
):
    global _last_results
    import os

    node_feat = np.ascontiguousarray(np.asarray(node_feat, dtype=np.float32))
    adj = np.asarray(adj_matrix, dtype=np.float32)

    # ---- host-side sharding / preprocessing ----
    deg = adj.sum(axis=1, dtype=np.float32) + 1.0
    dinv = (1.0 / np.sqrt(deg)).astype(np.float32)

    fp8 = ml_dtypes.float8_e4m3
    f32c = lambda a, shape=None: np.ascontiguousarray(
        np.asarray(a, dtype=np.float32).reshape(shape)
        if shape is not None
        else np.asarray(a, dtype=np.float32)
    )

    bf16 = ml_dtypes.bfloat16
    common = {
        "xt": np.ascontiguousarray(node_feat.T).astype(bf16),
        "w0": np.ascontiguousarray(np.asarray(W0, np.float32)).astype(bf16),
        "b0": f32c(b0, (-1, 1)),
        "w1": f32c(W1),
        "b1": f32c(b1, (-1, 1)),
        "w2": f32c(W2),
        "b2": f32c(b2, (-1, 1)),
        "wh1": f32c(Wh1),
        "bh1": f32c(bh1, (-1, 1)),
        "wh2": f32c(Wh2),
        "bh2": f32c(bh2, (-1, 1)),
    }

    in_maps = []
    idx = np.arange(ROWS)
    sdinv = dinv * np.float32(ASCALE)
    for k in range(NCORES):
        sl = slice(k * ROWS, (k + 1) * ROWS)
        # rows of ASCALE*Ahat for this core's output nodes
        blk = adj[sl, :] * sdinv[sl, None]
        blk *= dinv[None, :]
        blk[idx, k * ROWS + idx] = sdinv[sl] * dinv[sl]  # + I self loops
        a_k = blk.T.astype(fp8)  # [N, ROWS] = scaled Ahat.T cols, contiguous
        m = {"a_t": a_k}
        m.update(common)
        in_maps.append(m)

    from concourse import bass_utils

    nc = _get_nc()
    trace = bool(int(os.environ.get("GCN_TRACE", "0")))
    if trace:
        _install_ntff_hook()
    res = bass_utils.run_bass_kernel_spmd(
        nc, in_maps, core_ids=list(range(NCORES)), trace=trace
    )
    _last_results = res

    out0 = res.results[0]
    logits = np.asarray(out0["logits"], dtype=np.float32).reshape(2)
    probs = np.asarray(out0["probs"], dtype=np.float32).reshape(2)
    return (logits, probs)


# revision 16
# speedup vs baseline: 1.8082x; 1.0149x over previous
"""Trainium2 Bass kernel for a 3-layer GCN + mean-pool + MLP + softmax.

Reference computation (N=16384 nodes, dense adjacency):
    Ahat = D^-1/2 (A + I) D^-1/2
    H0 = X;  H_{l+1} = relu(Ahat @ (H_l @ W_l) + b_l)   l = 0,1,2
    g = mean(H3, axis=0);  h1 = elu(g @ Wh1 + bh1)
    logits = h1 @ Wh2 + bh2;  probs = softmax(logits)

Distribution (8 NeuronCores, 1D node/row parallel):
  - Host folds the symmetric degree normalization into the adjacency and
    ships each core the *transposed* normalized adjacency columns for its
    2048 output nodes as fp8 e4m3 (32MB/core), pre-tiled to the SBUF
    layout [group, partition, stripe, i] so every adjacency DMA reads
    per-partition-contiguous 16KB runs.  ASCALE=16 keeps entries in fp8
    normal range; it is divided back out by the relu activation's scale.
  - On device, the big matmul per layer streams the adjacency through the
    tensor engine (moving operand, DoubleRow fp8: 256-deep contraction)
    against the stationary Y_l = H_l @ W_l tiles:
        out.T[c, i] = sum_j Y_l[j, c] * Ahat.T[j, i]   (PSUM fp32 accum)
  - Y0 = X @ W0 is computed fully replicated on every core (X is tiny),
    so no collective is needed before layer 1 and the collective entry
    barrier overlaps with the adjacency stream.
  - Between layers: each core computes Y_{l+1} rows for its own nodes
    with a small fp32 matmul, then TWO half-node AllGathers replicate
    Y_{l+1}; the j-loop is ordered half-major so the second gather hides
    behind the first half's matmuls.
  - Mean pool: per-core partial sum over the free axis + AllReduce, then a
    replicated tiny MLP + softmax; core 0's output is returned.
  - DMA ring split: the bulk adjacency stream runs on the SP (nc.sync)
    HWDGE ring; all small loads that may wait on collectives run on the
    ACT (nc.scalar) ring so they never stall the adjacency stream.
"""

import numpy as np
import ml_dtypes

N = 16384
NCORES = 8
ROWS = N // NCORES          # 2048 output nodes per core
P = 128
DIMS = [64, 32, 48, 64]     # feature dims: in, after l0, l1, l2
NSTRIPE = 8                 # 128-row j-stripes per DMA group (2MB fp8)
NHALF = 2                   # half-node split for pipelined AllGathers
NGROUPS = NCORES * NHALF    # 16 groups per layer: (h, r)
QCH = 512                   # moving-operand free-dim chunk (1 PSUM bank)
NQ = ROWS // QCH            # 4
NU = ROWS // P              # 16 local node tiles
NDT = NSTRIPE // 2          # 4 double j-tiles per group (DoubleRow)
ASCALE = 16.0               # fp8 range helper, divided out in the relu
ABUFS = 8                   # adjacency groups in flight (16MB)

_nc_cache = None


def _build_nc():
    from concourse import bacc, mybir, tile

    dt = mybir.dt
    F32 = dt.float32
    F8 = dt.float8e4
    BF16 = dt.bfloat16
    AF = mybir.ActivationFunctionType
    OP = mybir.AluOpType
    DR = mybir.MatmulPerfMode.DoubleRow

    nc = bacc.Bacc(
        "TRN2", target_bir_lowering=False, debug=False, num_devices=NCORES
    )

    # adjacency pre-tiled on host: [h, r, p, t, i]
    a_t = nc.dram_tensor(
        "a_t", [NHALF, NCORES, P, NSTRIPE, ROWS], F8, kind="ExternalInput"
    )
    # full X.T, replicated: every core computes the whole Y0 = X @ W0 locally
    xt = nc.dram_tensor("xt", [DIMS[0], N], BF16, kind="ExternalInput")
    w_d = [
        nc.dram_tensor(
            f"w{l}", [DIMS[l], DIMS[l + 1]], BF16 if l == 0 else F32,
            kind="ExternalInput",
        )
        for l in range(3)
    ]
    b_d = [
        nc.dram_tensor(f"b{l}", [DIMS[l + 1], 1], F32, kind="ExternalInput")
        for l in range(3)
    ]
    wh1_d = nc.dram_tensor("wh1", [DIMS[3], 32], F32, kind="ExternalInput")
    bh1_d = nc.dram_tensor("bh1", [32, 1], F32, kind="ExternalInput")
    wh2_d = nc.dram_tensor("wh2", [32, 2], F32, kind="ExternalInput")
    bh2_d = nc.dram_tensor("bh2", [2, 1], F32, kind="ExternalInput")
    logits_o = nc.dram_tensor("logits", [2, 1], F32, kind="ExternalOutput")
    probs_o = nc.dram_tensor("probs", [2, 1], F32, kind="ExternalOutput")

    rg = [list(range(NCORES))]

    with tile.TileContext(nc) as tc:
        with (
            tc.tile_pool(name="const", bufs=1) as const,
            tc.tile_pool(name="apool", bufs=ABUFS) as apool,
            tc.tile_pool(name="spool", bufs=2) as spool,
            tc.tile_pool(name="hpool", bufs=2) as hpool,
            tc.tile_pool(name="ypool", bufs=2) as ypool,
            tc.tile_pool(name="xpool", bufs=2) as xpool,
            tc.tile_pool(name="smal", bufs=1) as smal,
            tc.tile_pool(name="accp", bufs=1, space="PSUM") as accp,
            tc.tile_pool(name="psml", bufs=2, space="PSUM") as psml,
            tc.tile_pool(name="psmlp", bufs=1, space="PSUM") as psmlp,
            tc.tile_pool(name="dram", bufs=1, space="DRAM") as dram,
        ):
            # ---- constants into SBUF (ACT ring — keep SP ring for A) ----
            def load(handle, shape, name, dtype=F32):
                t = const.tile(shape, dtype, name=name)
                nc.scalar.dma_start(t[:], handle.ap())
                return t

            w_sb = [
                load(
                    w_d[l], [DIMS[l], DIMS[l + 1]], f"w{l}sb",
                    dtype=BF16 if l == 0 else F32,
                )
                for l in range(3)
            ]
            b_sb = [load(b_d[l], [DIMS[l + 1], 1], f"b{l}sb") for l in range(3)]
            wh1_sb = load(wh1_d, [DIMS[3], 32], "wh1sb")
            bh1_sb = load(bh1_d, [32, 1], "bh1sb")
            wh2_sb = load(wh2_d, [32, 2], "wh2sb")
            bh2_sb = load(bh2_d, [2, 1], "bh2sb")

            # ---- layer-0 stationary: full Y0 = X @ W0, computed locally.
            # stat layout per half: [p, r, u8, c] with node j = (r*2 + h)*1024
            # + u8*128 + p;  j-tile jt = (h, r, u8).
            def stat_pair(l, c_out):
                return [
                    spool.tile(
                        [P, NCORES, NU // 2, c_out], F8,
                        tag=f"stat{h}", name=f"stat{l}_{h}",
                    )
                    for h in range(NHALF)
                ]

            stat = stat_pair(0, DIMS[1])
            for ck in range(8):  # X.T chunks of 2048 nodes
                xc = xpool.tile([DIMS[0], 2048], BF16, tag="xc", name=f"xc{ck}")
                nc.scalar.dma_start(xc[:], xt.ap()[:, ck * 2048 : (ck + 1) * 2048])
                for q in range(16):  # 16 j-tiles per chunk
                    jt = ck * 16 + q
                    r, h, u8 = jt // NU, (jt % NU) // 8, jt % 8
                    ps = psml.tile([P, DIMS[1]], F32, tag="psy", name=f"psy0_{jt}")
                    nc.tensor.matmul(
                        ps[:],
                        lhsT=xc[:, q * P : (q + 1) * P],
                        rhs=w_sb[0][:],
                        start=True,
                        stop=True,
                    )
                    nc.any.tensor_copy(out=stat[h][:, r, u8, :], in_=ps[:])

            h_sb = None
            for l in range(3):
                c_out = DIMS[l + 1]
                acc = [
                    accp.tile([P, QCH], F32, tag=f"acc{q}", name=f"acc{l}_{q}")
                    for q in range(NQ)
                ]
                gi = 0
                for h in range(NHALF):
                    for r in range(NCORES):
                        a_sb = apool.tile(
                            [P, NSTRIPE, ROWS], F8, tag="a", name=f"a{l}_{h}_{r}"
                        )
                        nc.sync.dma_start(a_sb[:], a_t.ap()[h, r])
                        for t2 in range(NDT):
                            lw = stat[h][:, r, 2 * t2 : 2 * t2 + 2, :]
                            for q in range(NQ):
                                nc.tensor.matmul(
                                    acc[q][:c_out, :],
                                    lhsT=lw,
                                    rhs=a_sb[
                                        :, 2 * t2 : 2 * t2 + 2,
                                        q * QCH : (q + 1) * QCH,
                                    ],
                                    start=(gi == 0 and t2 == 0),
                                    stop=(gi == NGROUPS - 1 and t2 == NDT - 1),
                                    perf_mode=DR,
                                )
                        gi += 1
                h_sb = hpool.tile([c_out, ROWS], F32, tag="h", name=f"h{l}")
                for q in range(NQ):
                    nc.scalar.activation(
                        h_sb[:, q * QCH : (q + 1) * QCH],
                        acc[q][:c_out, :],
                        AF.Relu,
                        bias=b_sb[l][:],
                        scale=1.0 / ASCALE,
                    )
                if l == 2:
                    break

                # ---- project local Y_{l+1} rows + two pipelined AllGathers
                c_next = DIMS[l + 2]
                stat = stat_pair(l + 1, c_next)
                for h in range(NHALF):
                    y_sb = ypool.tile(
                        [P, NU // 2, c_next], F8, tag="y", name=f"y{l}_{h}"
                    )
                    for u8 in range(NU // 2):
                        u = h * 8 + u8
                        ps = psml.tile(
                            [P, c_next], F32, tag="psy", name=f"psy{l}_{u}"
                        )
                        nc.tensor.matmul(
                            ps[:],
                            lhsT=h_sb[:, u * P : (u + 1) * P],
                            rhs=w_sb[l + 1][:],
                            start=True,
                            stop=True,
                        )
                        nc.any.tensor_copy(out=y_sb[:, u8, :], in_=ps[:])
                    ag_in = dram.tile(
                        [P, NU // 2, c_next], F8, name=f"agin{l}_{h}"
                    )
                    ag_out = dram.tile(
                        [NCORES, P, NU // 2, c_next], F8, name=f"agout{l}_{h}",
                        addr_space="Shared",
                    )
                    nc.scalar.dma_start(ag_in[:], y_sb[:])
                    nc.gpsimd.collective_compute(
                        "AllGather",
                        OP.bypass,
                        replica_groups=rg,
                        ins=[ag_in[:].opt()],
                        outs=[ag_out[:].opt()],
                    )
                    nc.scalar.dma_start(
                        stat[h][:], ag_out[:].rearrange("r p u c -> p r u c")
                    )

            # ---- mean pool over all nodes ----
            gp = smal.tile([DIMS[3], 1], F32, name="gpart")
            nc.vector.tensor_reduce(
                gp[:], h_sb[:], axis=mybir.AxisListType.X, op=OP.add
            )
            ar_in = dram.tile([DIMS[3], 1], F32, name="arin")
            ar_out = dram.tile([DIMS[3], 1], F32, name="arout", addr_space="Shared")
            nc.scalar.dma_start(ar_in[:], gp[:])
            nc.gpsimd.collective_compute(
                "AllReduce",
                OP.add,
                replica_groups=rg,
                ins=[ar_in[:].opt()],
                outs=[ar_out[:].opt()],
            )
            g_sb = smal.tile([DIMS[3], 1], F32, name="gsb")
            nc.scalar.dma_start(g_sb[:], ar_out[:])
            nc.any.tensor_scalar_mul(g_sb[:], g_sb[:], 1.0 / N)

            # ---- MLP head: h1 = elu(g @ Wh1 + bh1) ----
            ps1 = psmlp.tile([32, 1], F32, tag="mlp", name="ps1")
            nc.tensor.matmul(ps1[:], lhsT=wh1_sb[:], rhs=g_sb[:], start=True, stop=True)
            # elu(x) = relu(x) + exp(min(x, 0)) - 1
            tmin = smal.tile([32, 1], F32, name="tmin")
            nc.vector.tensor_scalar(tmin[:], ps1[:], bh1_sb[:], 0.0, OP.add, OP.min)
            e1 = smal.tile([32, 1], F32, name="e1")
            nc.scalar.activation(e1[:], tmin[:], AF.Exp)
            r1 = smal.tile([32, 1], F32, name="r1")
            nc.scalar.activation(r1[:], ps1[:], AF.Relu, bias=bh1_sb[:])
            h1 = smal.tile([32, 1], F32, name="h1")
            nc.vector.tensor_tensor(h1[:], e1[:], r1[:], OP.add)
            nc.vector.tensor_scalar_add(h1[:], h1[:], -1.0)

            # ---- logits = h1 @ Wh2 + bh2; probs = softmax(logits) ----
            ps2 = psmlp.tile([2, 1], F32, tag="mlp", name="ps2")
            nc.tensor.matmul(ps2[:], lhsT=wh2_sb[:], rhs=h1[:], start=True, stop=True)
            logit_sb = smal.tile([2, 1], F32, name="logitsb")
            nc.vector.tensor_scalar(logit_sb[:], ps2[:], bh2_sb[:], None, OP.add)
            nc.scalar.dma_start(logits_o.ap(), logit_sb[:])

            e2 = smal.tile([2, 1], F32, name="e2")
            nc.scalar.activation(e2[:], logit_sb[:], AF.Exp)
            ones21 = smal.tile([2, 1], F32, name="ones21")
            nc.any.memset(ones21[:], 1.0)
            ones12 = smal.tile([1, 2], F32, name="ones12")
            nc.any.memset(ones12[:], 1.0)
            ps3 = psmlp.tile([1, 1], F32, tag="mlp", name="ps3")
            nc.tensor.matmul(ps3[:], lhsT=e2[:], rhs=ones21[:], start=True, stop=True)
            rs = smal.tile([1, 1], F32, name="rs")
            nc.vector.reciprocal(rs[:], ps3[:])
            ps4 = psmlp.tile([2, 1], F32, tag="mlp", name="ps4")
            nc.tensor.matmul(ps4[:], lhsT=ones12[:], rhs=rs[:], start=True, stop=True)
            probs_sb = smal.tile([2, 1], F32, name="probssb")
            nc.vector.tensor_tensor(probs_sb[:], e2[:], ps4[:], OP.mult)
            nc.scalar.dma_start(probs_o.ap(), probs_sb[:])

    nc.finalize()
    return nc


def _install_ntff_hook():
    """Register the axon NTFF profiling hook if the container's antenv stub
    lacks it (bass_utils imports antenv.axon_hooks when trace=True)."""
    import sys
    import types

    try:
        import antenv.axon_hooks  # noqa: F401
        return
    except ImportError:
        pass
    mod = types.ModuleType("antenv.axon_hooks")
    _h = [None]
    mod.set_axon_ntff_profile_hook = lambda h: _h.__setitem__(0, h)
    mod.get_axon_ntff_profile_hook = lambda: _h[0]
    sys.modules["antenv.axon_hooks"] = mod
    import antenv

    antenv.axon_hooks = mod
    try:
        from trn_agent_boot import trn_boot

        hook = trn_boot._ntff_profile_via_ctypes("/opt/axon/libaxon_pjrt.so")
        if hook is not None:
            mod.set_axon_ntff_profile_hook(hook)
    except Exception:
        pass


def _get_nc():
    global _nc_cache
    if _nc_cache is None:
        _nc_cache = _build_nc()
    return _nc_cache


_last_results = None


def kernel(
    node_feat,
    adj_matrix,
    W0,
    b0,
    W1,
    b1,
    W2,
    b2,
    Wh1,
    bh1,
    Wh2,
    bh2,
):
    global _last_results
    import os

    node_feat = np.ascontiguousarray(np.asarray(node_feat, dtype=np.float32))
    adj = np.asarray(adj_matrix, dtype=np.float32)

    # ---- host-side sharding / preprocessing ----
    deg = adj.sum(axis=1, dtype=np.float32) + 1.0
    dinv = (1.0 / np.sqrt(deg)).astype(np.float32)

    fp8 = ml_dtypes.float8_e4m3
    bf16 = ml_dtypes.bfloat16
    f32c = lambda a, shape=None: np.ascontiguousarray(
        np.asarray(a, dtype=np.float32).reshape(shape)
        if shape is not None
        else np.asarray(a, dtype=np.float32)
    )

    common = {
        "xt": np.ascontiguousarray(node_feat.T).astype(bf16),
        "w0": np.ascontiguousarray(np.asarray(W0, np.float32)).astype(bf16),
        "b0": f32c(b0, (-1, 1)),
        "w1": f32c(W1),
        "b1": f32c(b1, (-1, 1)),
        "w2": f32c(W2),
        "b2": f32c(b2, (-1, 1)),
        "wh1": f32c(Wh1),
        "bh1": f32c(bh1, (-1, 1)),
        "wh2": f32c(Wh2),
        "bh2": f32c(bh2, (-1, 1)),
    }

    in_maps = []
    idx = np.arange(ROWS)
    sdinv = dinv * np.float32(ASCALE)
    for k in range(NCORES):
        sl = slice(k * ROWS, (k + 1) * ROWS)
        # rows of ASCALE*Ahat for this core's output nodes
        blk = adj[sl, :] * sdinv[sl, None]
        blk *= dinv[None, :]
        blk[idx, k * ROWS + idx] = sdinv[sl] * dinv[sl]  # + I self loops
        a_k = blk.T.astype(fp8)  # [N, ROWS] = scaled Ahat.T cols
        # pre-tile to device layout [h, r, p, t, i]:
        # row j = r*2048 + h*1024 + t*128 + p
        a_k = np.ascontiguousarray(
            a_k.reshape(NCORES, NHALF, NSTRIPE, P, ROWS).transpose(1, 0, 3, 2, 4)
        )
        m = {"a_t": a_k}
        m.update(common)
        in_maps.append(m)

    from concourse import bass_utils

    nc = _get_nc()
    trace = bool(int(os.environ.get("GCN_TRACE", "0")))
    if trace:
        _install_ntff_hook()
    res = bass_utils.run_bass_kernel_spmd(
        nc, in_maps, core_ids=list(range(NCORES)), trace=trace
    )
    _last_results = res

    out0 = res.results[0]
    logits = np.asarray(out0["logits"], dtype=np.float32).reshape(2)
    probs = np.asarray(out0["probs"], dtype=np.float32).reshape(2)
    return (logits, probs)


# revision 18
# speedup vs baseline: 1.9225x; 1.0632x over previous
"""Trainium2 Bass kernel for a 3-layer GCN + mean-pool + MLP + softmax.

Reference computation (N=16384 nodes, dense adjacency):
    Ahat = D^-1/2 (A + I) D^-1/2
    H0 = X;  H_{l+1} = relu(Ahat @ (H_l @ W_l) + b_l)   l = 0,1,2
    g = mean(H3, axis=0);  h1 = elu(g @ Wh1 + bh1)
    logits = h1 @ Wh2 + bh2;  probs = softmax(logits)

Distribution (8 NeuronCores, 1D node/row parallel):
  - Host folds the symmetric degree normalization into the adjacency and
    ships each core the *transposed* normalized adjacency columns for its
    2048 output nodes as fp8 e4m3 (32MB/core), pre-tiled to the SBUF
    layout [half, rank, partition, stripe, i] so every adjacency DMA
    reads per-partition-contiguous runs.  ASCALE/XSCALE keep fp8 values
    in normal range and are divided back out by the relu's scale.
  - On device, the big matmul per layer streams the adjacency through the
    tensor engine (moving operand, DoubleRow fp8: 256-deep contraction)
    against stationary Y_l = H_l @ W_l tiles:
        out.T[c, i] = sum_j Y_l[j, c] * Ahat.T[j, i]   (PSUM fp32 accum)
  - Layer 1 uses associativity: Ahat @ (X W0) = (Ahat @ X) W0, with X
    itself (fp8, host-tiled) as the stationary — no device-side Y0 and no
    collective before layer 1, so the collective entry barrier and rank
    skew hide behind the adjacency stream.
  - Between layers: each core computes Y_{l+1} rows for its own nodes
    with a small fp32 matmul, then TWO half-node AllGathers replicate
    Y_{l+1}; the j-loop is ordered half-major so the second gather hides
    behind the first half's matmuls.  A tiny background AllReduce fires
    mid-layer so cross-core skew is absorbed on the CC cores instead of
    at the AllGather.
  - Mean pool: per-core partial sum over the free axis + AllReduce, then a
    replicated tiny MLP + softmax; core 0's output is returned.
  - DMA ring split: the bulk adjacency stream runs on the SP (nc.sync)
    HWDGE ring; all small loads that may wait on collectives run on the
    ACT (nc.scalar) ring so they never stall the adjacency stream.
"""

import numpy as np
import ml_dtypes

N = 16384
NCORES = 8
ROWS = N // NCORES          # 2048 output nodes per core
P = 128
DIMS = [64, 32, 48, 64]     # feature dims: in, after l0, l1, l2
NSTRIPE = 8                 # 128-row j-stripes per DMA group (2MB fp8)
NHALF = 2                   # half-node split for pipelined AllGathers
NGROUPS = NCORES * NHALF    # 16 groups per layer: (h, r)
QCH = 512                   # moving-operand free-dim chunk (1 PSUM bank)
NQ = ROWS // QCH            # 4
NU = ROWS // P              # 16 local node tiles
NDT = NSTRIPE // 2          # 4 double j-tiles per group (DoubleRow)
ASCALE = 16.0               # fp8 range helper for Ahat
XSCALE = 16.0               # fp8 range helper for X
ABUFS = 8                   # adjacency groups in flight (16MB)

_nc_cache = None


def _build_nc():
    from concourse import bacc, mybir, tile

    dt = mybir.dt
    F32 = dt.float32
    F8 = dt.float8e4
    AF = mybir.ActivationFunctionType
    OP = mybir.AluOpType
    DR = mybir.MatmulPerfMode.DoubleRow

    nc = bacc.Bacc(
        "TRN2", target_bir_lowering=False, debug=False, num_devices=NCORES
    )

    # adjacency pre-tiled on host: [h, r, p, t, i]
    a_t = nc.dram_tensor(
        "a_t", [NHALF, NCORES, P, NSTRIPE, ROWS], F8, kind="ExternalInput"
    )
    # full X (scaled, fp8), pre-tiled like a stationary: [h, r, p, u8, c]
    x8 = nc.dram_tensor(
        "x8", [NHALF, NCORES, P, NU // 2, DIMS[0]], F8, kind="ExternalInput"
    )
    w_d = [
        nc.dram_tensor(f"w{l}", [DIMS[l], DIMS[l + 1]], F32, kind="ExternalInput")
        for l in range(3)
    ]
    b_d = [
        nc.dram_tensor(f"b{l}", [DIMS[l + 1], 1], F32, kind="ExternalInput")
        for l in range(3)
    ]
    wh1_d = nc.dram_tensor("wh1", [DIMS[3], 32], F32, kind="ExternalInput")
    bh1_d = nc.dram_tensor("bh1", [32, 1], F32, kind="ExternalInput")
    wh2_d = nc.dram_tensor("wh2", [32, 2], F32, kind="ExternalInput")
    bh2_d = nc.dram_tensor("bh2", [2, 1], F32, kind="ExternalInput")
    logits_o = nc.dram_tensor("logits", [2, 1], F32, kind="ExternalOutput")
    probs_o = nc.dram_tensor("probs", [2, 1], F32, kind="ExternalOutput")

    rg = [list(range(NCORES))]

    with tile.TileContext(nc) as tc:
        with (
            tc.tile_pool(name="const", bufs=1) as const,
            tc.tile_pool(name="apool", bufs=ABUFS) as apool,
            tc.tile_pool(name="spool", bufs=2) as spool,
            tc.tile_pool(name="hpool", bufs=2) as hpool,
            tc.tile_pool(name="ypool", bufs=2) as ypool,
            tc.tile_pool(name="smal", bufs=1) as smal,
            tc.tile_pool(name="accp", bufs=1, space="PSUM") as accp,
            tc.tile_pool(name="psml", bufs=3, space="PSUM") as psml,
            tc.tile_pool(name="psmlp", bufs=1, space="PSUM") as psmlp,
            tc.tile_pool(name="dram", bufs=1, space="DRAM") as dram,
        ):
            # ---- constants into SBUF (ACT ring — keep SP ring for A) ----
            def load(handle, shape, name, dtype=F32):
                t = const.tile(shape, dtype, name=name)
                nc.scalar.dma_start(t[:], handle.ap())
                return t

            w_sb = [
                load(w_d[l], [DIMS[l], DIMS[l + 1]], f"w{l}sb") for l in range(3)
            ]
            b_sb = [load(b_d[l], [DIMS[l + 1], 1], f"b{l}sb") for l in range(3)]
            wh1_sb = load(wh1_d, [DIMS[3], 32], "wh1sb")
            bh1_sb = load(bh1_d, [32, 1], "bh1sb")
            wh2_sb = load(wh2_d, [32, 2], "wh2sb")
            bh2_sb = load(bh2_d, [2, 1], "bh2sb")

            # background-resync collective plumbing (absorbs rank skew on
            # the CC cores, concurrently with compute)
            rs_sb = smal.tile([1, 1], F32, name="rssb")
            nc.any.memset(rs_sb[:], 1.0)
            rs_in = dram.tile([1, 1], F32, name="rsin")
            nc.scalar.dma_start(rs_in[:], rs_sb[:])

            def resync(tag):
                rs_out = dram.tile([1, 1], F32, name=f"rsout_{tag}")
                nc.gpsimd.collective_compute(
                    "AllReduce",
                    OP.add,
                    replica_groups=rg,
                    ins=[rs_in[:].opt()],
                    outs=[rs_out[:].opt()],
                )

            # ---- layer-1 stationary = X itself (fp8, host-tiled) ----
            def stat_pair(l, c_out):
                return [
                    spool.tile(
                        [P, NCORES, NU // 2, c_out], F8,
                        tag=f"stat{h}", name=f"stat{l}_{h}",
                    )
                    for h in range(NHALF)
                ]

            stat = stat_pair(0, DIMS[0])
            for h in range(NHALF):
                nc.scalar.dma_start(
                    stat[h][:], x8.ap()[h].rearrange("r p u c -> p r u c")
                )

            h_sb = None
            for l in range(3):
                c_stat = DIMS[0] if l == 0 else DIMS[l + 1]
                c_out = DIMS[l + 1]
                acc = [
                    accp.tile([P, QCH], F32, tag=f"acc{q}", name=f"acc{l}_{q}")
                    for q in range(NQ)
                ]
                gi = 0
                for h in range(NHALF):
                    for r in range(NCORES):
                        a_sb = apool.tile(
                            [P, NSTRIPE, ROWS], F8, tag="a", name=f"a{l}_{h}_{r}"
                        )
                        nc.sync.dma_start(a_sb[:], a_t.ap()[h, r])
                        for t2 in range(NDT):
                            lw = stat[h][:, r, 2 * t2 : 2 * t2 + 2, :]
                            for q in range(NQ):
                                nc.tensor.matmul(
                                    acc[q][:c_stat, :],
                                    lhsT=lw,
                                    rhs=a_sb[
                                        :, 2 * t2 : 2 * t2 + 2,
                                        q * QCH : (q + 1) * QCH,
                                    ],
                                    start=(gi == 0 and t2 == 0),
                                    stop=(gi == NGROUPS - 1 and t2 == NDT - 1),
                                    perf_mode=DR,
                                )
                        gi += 1
                        if h == 1 and r == 1:
                            resync(f"rs{l}")

                h_sb = hpool.tile([c_out, ROWS], F32, tag="h", name=f"h{l}")
                for q in range(NQ):
                    if l == 0:
                        # H1 chunk = relu((Ahat@X)chunk @ W0 / s + b0)
                        p1 = ypool.tile(
                            [DIMS[0], QCH], F32, tag="p1", name=f"p1_{q}"
                        )
                        nc.vector.tensor_copy(out=p1[:], in_=acc[q][: DIMS[0], :])
                        ps2 = psml.tile(
                            [DIMS[1], QCH], F32, tag="psy", name=f"ps2_{q}"
                        )
                        nc.tensor.matmul(
                            ps2[:], lhsT=w_sb[0][:], rhs=p1[:],
                            start=True, stop=True,
                        )
                        nc.scalar.activation(
                            h_sb[:, q * QCH : (q + 1) * QCH],
                            ps2[:],
                            AF.Relu,
                            bias=b_sb[0][:],
                            scale=1.0 / (ASCALE * XSCALE),
                        )
                    else:
                        nc.scalar.activation(
                            h_sb[:, q * QCH : (q + 1) * QCH],
                            acc[q][:c_out, :],
                            AF.Relu,
                            bias=b_sb[l][:],
                            scale=1.0 / ASCALE,
                        )
                if l == 2:
                    break

                # ---- project local Y_{l+1} rows + two pipelined AllGathers
                c_next = DIMS[l + 2]
                stat = stat_pair(l + 1, c_next)
                for h in range(NHALF):
                    y_sb = ypool.tile(
                        [P, NU // 2, c_next], F8, tag="y", name=f"y{l}_{h}"
                    )
                    for u8 in range(NU // 2):
                        u = h * 8 + u8
                        ps = psml.tile(
                            [P, c_next], F32, tag="psy", name=f"psy{l}_{u}"
                        )
                        nc.tensor.matmul(
                            ps[:],
                            lhsT=h_sb[:, u * P : (u + 1) * P],
                            rhs=w_sb[l + 1][:],
                            start=True,
                            stop=True,
                        )
                        nc.vector.tensor_copy(out=y_sb[:, u8, :], in_=ps[:])
                    ag_in = dram.tile(
                        [P, NU // 2, c_next], F8, name=f"agin{l}_{h}"
                    )
                    ag_out = dram.tile(
                        [NCORES, P, NU // 2, c_next], F8, name=f"agout{l}_{h}",
                        addr_space="Shared",
                    )
                    nc.scalar.dma_start(ag_in[:], y_sb[:])
                    nc.gpsimd.collective_compute(
                        "AllGather",
                        OP.bypass,
                        replica_groups=rg,
                        ins=[ag_in[:].opt()],
                        outs=[ag_out[:].opt()],
                    )
                    nc.scalar.dma_start(
                        stat[h][:], ag_out[:].rearrange("r p u c -> p r u c")
                    )

            # ---- mean pool over all nodes ----
            gp = smal.tile([DIMS[3], 1], F32, name="gpart")
            nc.vector.tensor_reduce(
                gp[:], h_sb[:], axis=mybir.AxisListType.X, op=OP.add
            )
            ar_in = dram.tile([DIMS[3], 1], F32, name="arin")
            ar_out = dram.tile([DIMS[3], 1], F32, name="arout", addr_space="Shared")
            nc.scalar.dma_start(ar_in[:], gp[:])
            nc.gpsimd.collective_compute(
                "AllReduce",
                OP.add,
                replica_groups=rg,
                ins=[ar_in[:].opt()],
                outs=[ar_out[:].opt()],
            )
            g_sb = smal.tile([DIMS[3], 1], F32, name="gsb")
            nc.scalar.dma_start(g_sb[:], ar_out[:])
            nc.any.tensor_scalar_mul(g_sb[:], g_sb[:], 1.0 / N)

            # ---- MLP head: h1 = elu(g @ Wh1 + bh1) ----
            ps1 = psmlp.tile([32, 1], F32, tag="mlp", name="ps1")
            nc.tensor.matmul(ps1[:], lhsT=wh1_sb[:], rhs=g_sb[:], start=True, stop=True)
            # elu(x) = relu(x) + exp(min(x, 0)) - 1
            tmin = smal.tile([32, 1], F32, name="tmin")
            nc.vector.tensor_scalar(tmin[:], ps1[:], bh1_sb[:], 0.0, OP.add, OP.min)
            e1 = smal.tile([32, 1], F32, name="e1")
            nc.scalar.activation(e1[:], tmin[:], AF.Exp)
            r1 = smal.tile([32, 1], F32, name="r1")
            nc.scalar.activation(r1[:], ps1[:], AF.Relu, bias=bh1_sb[:])
            h1 = smal.tile([32, 1], F32, name="h1")
            nc.vector.tensor_tensor(h1[:], e1[:], r1[:], OP.add)
            nc.vector.tensor_scalar_add(h1[:], h1[:], -1.0)

            # ---- logits = h1 @ Wh2 + bh2; probs = softmax(logits) ----
            ps2m = psmlp.tile([2, 1], F32, tag="mlp", name="ps2m")
            nc.tensor.matmul(ps2m[:], lhsT=wh2_sb[:], rhs=h1[:], start=True, stop=True)
            logit_sb = smal.tile([2, 1], F32, name="logitsb")
            nc.vector.tensor_scalar(logit_sb[:], ps2m[:], bh2_sb[:], None, OP.add)
            nc.scalar.dma_start(logits_o.ap(), logit_sb[:])

            e2 = smal.tile([2, 1], F32, name="e2")
            nc.scalar.activation(e2[:], logit_sb[:], AF.Exp)
            ones21 = smal.tile([2, 1], F32, name="ones21")
            nc.any.memset(ones21[:], 1.0)
            ones12 = smal.tile([1, 2], F32, name="ones12")
            nc.any.memset(ones12[:], 1.0)
            ps3 = psmlp.tile([1, 1], F32, tag="mlp", name="ps3")
            nc.tensor.matmul(ps3[:], lhsT=e2[:], rhs=ones21[:], start=True, stop=True)
            rsc = smal.tile([1, 1], F32, name="rsc")
            nc.vector.reciprocal(rsc[:], ps3[:])
            ps4 = psmlp.tile([2, 1], F32, tag="mlp", name="ps4")
            nc.tensor.matmul(ps4[:], lhsT=ones12[:], rhs=rsc[:], start=True, stop=True)
            probs_sb = smal.tile([2, 1], F32, name="probssb")
            nc.vector.tensor_tensor(probs_sb[:], e2[:], ps4[:], OP.mult)
            nc.scalar.dma_start(probs_o.ap(), probs_sb[:])

    nc.finalize()
    return nc


def _install_ntff_hook():
    """Register the axon NTFF profiling hook if the container's antenv stub
    lacks it (bass_utils imports antenv.axon_hooks when trace=True)."""
    import sys
    import types

    try:
        import antenv.axon_hooks  # noqa: F401
        return
    except ImportError:
        pass
    mod = types.ModuleType("antenv.axon_hooks")
    _h = [None]
    mod.set_axon_ntff_profile_hook = lambda h: _h.__setitem__(0, h)
    mod.get_axon_ntff_profile_hook = lambda: _h[0]
    sys.modules["antenv.axon_hooks"] = mod
    import antenv

    antenv.axon_hooks = mod
    try:
        from trn_agent_boot import trn_boot

        hook = trn_boot._ntff_profile_via_ctypes("/opt/axon/libaxon_pjrt.so")
        if hook is not None:
            mod.set_axon_ntff_profile_hook(hook)
    except Exception:
        pass


def _get_nc():
    global _nc_cache
    if _nc_cache is None:
        _nc_cache = _build_nc()
    return _nc_cache


_last_results = None


def kernel(
    node_feat,
    adj_matrix,
    W0,
    b0,
    W1,
    b1,
    W2,
    b2,
    Wh1,
    bh1,
    Wh2,
    bh2,
):
    global _last_results
    import os

    node_feat = np.ascontiguousarray(np.asarray(node_feat, dtype=np.float32))
    adj = np.asarray(adj_matrix, dtype=np.float32)

    # ---- host-side sharding / preprocessing ----
    deg = adj.sum(axis=1, dtype=np.float32) + 1.0
    dinv = (1.0 / np.sqrt(deg)).astype(np.float32)

    fp8 = ml_dtypes.float8_e4m3
    f32c = lambda a, shape=None: np.ascontiguousarray(
        np.asarray(a, dtype=np.float32).reshape(shape)
        if shape is not None
        else np.asarray(a, dtype=np.float32)
    )

    # X scaled to fp8, tiled [h, r, p, u8, c]: node j = r*2048+h*1024+u8*128+p
    x8 = (node_feat * np.float32(XSCALE)).astype(fp8)
    x8 = np.ascontiguousarray(
        x8.reshape(NCORES, NHALF, NU // 2, P, DIMS[0]).transpose(1, 0, 3, 2, 4)
    )

    common = {
        "x8": x8,
        "w0": f32c(W0),
        "b0": f32c(b0, (-1, 1)),
        "w1": f32c(W1),
        "b1": f32c(b1, (-1, 1)),
        "w2": f32c(W2),
        "b2": f32c(b2, (-1, 1)),
        "wh1": f32c(Wh1),
        "bh1": f32c(bh1, (-1, 1)),
        "wh2": f32c(Wh2),
        "bh2": f32c(bh2, (-1, 1)),
    }

    in_maps = []
    idx = np.arange(ROWS)
    sdinv = dinv * np.float32(ASCALE)
    for k in range(NCORES):
        sl = slice(k * ROWS, (k + 1) * ROWS)
        # rows of ASCALE*Ahat for this core's output nodes
        blk = adj[sl, :] * sdinv[sl, None]
        blk *= dinv[None, :]
        blk[idx, k * ROWS + idx] = sdinv[sl] * dinv[sl]  # + I self loops
        a_k = blk.T.astype(fp8)  # [N, ROWS] = scaled Ahat.T cols
        # pre-tile to device layout [h, r, p, t, i]:
        # row j = r*2048 + h*1024 + t*128 + p
        a_k = np.ascontiguousarray(
            a_k.reshape(NCORES, NHALF, NSTRIPE, P, ROWS).transpose(1, 0, 3, 2, 4)
        )
        m = {"a_t": a_k}
        m.update(common)
        in_maps.append(m)

    from concourse import bass_utils

    nc = _get_nc()
    trace = bool(int(os.environ.get("GCN_TRACE", "0")))
    if trace:
        _install_ntff_hook()
    res = bass_utils.run_bass_kernel_spmd(
        nc, in_maps, core_ids=list(range(NCORES)), trace=trace
    )
    _last_results = res

    out0 = res.results[0]
    logits = np.asarray(out0["logits"], dtype=np.float32).reshape(2)
    probs = np.asarray(out0["probs"], dtype=np.float32).reshape(2)
    return (logits, probs)


# revision 24
# speedup vs baseline: 2.0170x; 1.0492x over previous
"""Trainium2 Bass kernel for a 3-layer GCN + mean-pool + MLP + softmax.

Reference computation (N=16384 nodes, dense adjacency):
    Ahat = D^-1/2 (A + I) D^-1/2
    H0 = X;  H_{l+1} = relu(Ahat @ (H_l @ W_l) + b_l)   l = 0,1,2
    g = mean(H3, axis=0);  h1 = elu(g @ Wh1 + bh1)
    logits = h1 @ Wh2 + bh2;  probs = softmax(logits)

Distribution (8 NeuronCores, 1D node/row parallel):
  - Host folds the symmetric degree normalization into the adjacency and
    ships each core the *transposed* normalized adjacency columns for its
    2048 output nodes as fp8 e4m3 (32MB/core), pre-tiled to the SBUF
    layout [half, rank, partition, stripe, i] so every adjacency DMA
    reads per-partition-contiguous runs.  ASCALE/XSCALE keep fp8 values
    in normal range and are divided back out by the relu's scale.
  - On device, the big matmul per layer streams the adjacency through the
    tensor engine (moving operand, DoubleRow fp8: 256-deep contraction)
    against stationary Y_l = H_l @ W_l tiles:
        out.T[c, i] = sum_j Y_l[j, c] * Ahat.T[j, i]   (PSUM fp32 accum)
  - Layer 1 uses associativity: Ahat @ (X W0) = (Ahat @ X) W0, with X
    itself (fp8, host-tiled) as the stationary — no device-side Y0 and no
    collective before layer 1, so the collective entry barrier and rank
    skew hide behind the adjacency stream.
  - Between layers: each core computes Y_{l+1} rows for its own nodes
    with a small fp32 matmul, then TWO half-node AllGathers replicate
    Y_{l+1}; the j-loop is ordered half-major so the second gather hides
    behind the first half's matmuls.  A tiny background AllReduce fires
    mid-layer so cross-core skew is absorbed on the CC cores instead of
    at the AllGather.
  - Mean pool: per-core partial sum over the free axis + AllReduce, then a
    replicated tiny MLP + softmax; core 0's output is returned.
  - DMA ring split: the bulk adjacency stream runs on the SP (nc.sync)
    HWDGE ring; all small loads that may wait on collectives run on the
    ACT (nc.scalar) ring so they never stall the adjacency stream.
"""

import numpy as np
import ml_dtypes

N = 16384
NCORES = 8
ROWS = N // NCORES          # 2048 output nodes per core
P = 128
DIMS = [64, 32, 48, 64]     # feature dims: in, after l0, l1, l2
NSTRIPE = 8                 # 128-row j-stripes per DMA group (2MB fp8)
NHALF = 2                   # half-node split for pipelined AllGathers
NGROUPS = NCORES * NHALF    # 16 groups per layer: (h, r)
QCH = 512                   # moving-operand free-dim chunk (1 PSUM bank)
NQ = ROWS // QCH            # 4
NU = ROWS // P              # 16 local node tiles
NDT = NSTRIPE // 2          # 4 double j-tiles per group (DoubleRow)
ASCALE = 16.0               # fp8 range helper for Ahat
XSCALE = 16.0               # fp8 range helper for X
ABUFS = 8                   # adjacency groups in flight (16MB)

_nc_cache = None


def _build_nc():
    from concourse import bacc, mybir, tile

    dt = mybir.dt
    F32 = dt.float32
    F8 = dt.float8e4
    AF = mybir.ActivationFunctionType
    OP = mybir.AluOpType
    DR = mybir.MatmulPerfMode.DoubleRow

    nc = bacc.Bacc(
        "TRN2", target_bir_lowering=False, debug=False, num_devices=NCORES
    )

    # adjacency pre-tiled on host: [h, r, p, t, i]
    a_t = nc.dram_tensor(
        "a_t", [NHALF, NCORES, P, NSTRIPE, ROWS], F8, kind="ExternalInput"
    )
    # full X (scaled, fp8), pre-tiled partition-major so each partition's
    # stationary data is one contiguous run: [p, h, r, u8, c]
    x8 = nc.dram_tensor(
        "x8", [P, NHALF, NCORES, NU // 2, DIMS[0]], F8, kind="ExternalInput"
    )
    w_d = [
        nc.dram_tensor(f"w{l}", [DIMS[l], DIMS[l + 1]], F32, kind="ExternalInput")
        for l in range(3)
    ]
    b_d = [
        nc.dram_tensor(f"b{l}", [DIMS[l + 1], 1], F32, kind="ExternalInput")
        for l in range(3)
    ]
    wh1_d = nc.dram_tensor("wh1", [DIMS[3], 32], F32, kind="ExternalInput")
    bh1_d = nc.dram_tensor("bh1", [32, 1], F32, kind="ExternalInput")
    wh2_d = nc.dram_tensor("wh2", [32, 2], F32, kind="ExternalInput")
    bh2_d = nc.dram_tensor("bh2", [2, 1], F32, kind="ExternalInput")
    logits_o = nc.dram_tensor("logits", [2, 1], F32, kind="ExternalOutput")
    probs_o = nc.dram_tensor("probs", [2, 1], F32, kind="ExternalOutput")

    rg = [list(range(NCORES))]

    with tile.TileContext(nc) as tc:
        with (
            tc.tile_pool(name="const", bufs=1) as const,
            tc.tile_pool(name="apool", bufs=ABUFS) as apool,
            tc.tile_pool(name="spool", bufs=2) as spool,
            tc.tile_pool(name="hpool", bufs=2) as hpool,
            tc.tile_pool(name="ypool", bufs=2) as ypool,
            tc.tile_pool(name="smal", bufs=1) as smal,
            tc.tile_pool(name="accp", bufs=1, space="PSUM") as accp,
            tc.tile_pool(name="psml", bufs=3, space="PSUM") as psml,
            tc.tile_pool(name="psmlp", bufs=1, space="PSUM") as psmlp,
            tc.tile_pool(name="dram", bufs=1, space="DRAM") as dram,
        ):
            # ---- constants into SBUF (ACT ring — keep SP ring for A) ----
            def load(handle, shape, name, dtype=F32):
                t = const.tile(shape, dtype, name=name)
                nc.scalar.dma_start(t[:], handle.ap())
                return t

            w_sb = [
                load(w_d[l], [DIMS[l], DIMS[l + 1]], f"w{l}sb") for l in range(3)
            ]
            b_sb = [load(b_d[l], [DIMS[l + 1], 1], f"b{l}sb") for l in range(3)]
            wh1_sb = load(wh1_d, [DIMS[3], 32], "wh1sb")
            bh1_sb = load(bh1_d, [32, 1], "bh1sb")
            wh2_sb = load(wh2_d, [32, 2], "wh2sb")
            bh2_sb = load(bh2_d, [2, 1], "bh2sb")


            def resync(tag, dep_ap):
                # chain the trigger on `dep_ap` (a mid-layer adjacency tile)
                # so every rank fires this at the same point in its layer
                rs_src = smal.tile([1, 1], F32, name=f"rss_{tag}")
                nc.vector.tensor_copy(out=rs_src[:], in_=dep_ap)
                rs_in = dram.tile([1, 1], F32, name=f"rsin_{tag}")
                nc.scalar.dma_start(rs_in[:], rs_src[:])
                rs_out = dram.tile([1, 1], F32, name=f"rsout_{tag}")
                nc.gpsimd.collective_compute(
                    "AllReduce",
                    OP.add,
                    replica_groups=rg,
                    ins=[rs_in[:].opt()],
                    outs=[rs_out[:].opt()],
                )

            # ---- layer-1 stationary = X itself (fp8, host-tiled) ----
            def stat_pair(l, c_out):
                return [
                    spool.tile(
                        [P, NCORES, NU // 2, c_out], F8,
                        tag=f"stat{h}", name=f"stat{l}_{h}",
                    )
                    for h in range(NHALF)
                ]

            stat = stat_pair(0, DIMS[0])
            for h in range(NHALF):
                nc.scalar.dma_start(stat[h][:], x8.ap()[:, h])

            h_sb = None
            for l in range(3):
                c_stat = DIMS[0] if l == 0 else DIMS[l + 1]
                c_out = DIMS[l + 1]
                acc = [
                    accp.tile([P, QCH], F32, tag=f"acc{q}", name=f"acc{l}_{q}")
                    for q in range(NQ)
                ]
                gi = 0
                for h in range(NHALF):
                    for r in range(NCORES):
                        a_sb = apool.tile(
                            [P, NSTRIPE, ROWS], F8, tag="a", name=f"a{l}_{h}_{r}"
                        )
                        nc.sync.dma_start(a_sb[:], a_t.ap()[h, r])
                        for t2 in range(NDT):
                            lw = stat[h][:, r, 2 * t2 : 2 * t2 + 2, :]
                            for q in range(NQ):
                                nc.tensor.matmul(
                                    acc[q][:c_stat, :],
                                    lhsT=lw,
                                    rhs=a_sb[
                                        :, 2 * t2 : 2 * t2 + 2,
                                        q * QCH : (q + 1) * QCH,
                                    ],
                                    start=(gi == 0 and t2 == 0),
                                    stop=(gi == NGROUPS - 1 and t2 == NDT - 1),
                                    perf_mode=DR,
                                )
                        gi += 1
                        if h == 1 and r == (4 if l == 2 else 1):
                            resync(f"rs{l}", a_sb[0:1, 0, 0:1])

                h_sb = hpool.tile([c_out, ROWS], F32, tag="h", name=f"h{l}")
                for q in range(NQ):
                    if l == 0:
                        # H1 chunk = relu((Ahat@X)chunk @ W0 / s + b0)
                        p1 = ypool.tile(
                            [DIMS[0], QCH], F32, tag="p1", name=f"p1_{q}"
                        )
                        nc.vector.tensor_copy(out=p1[:], in_=acc[q][: DIMS[0], :])
                        ps2 = psml.tile(
                            [DIMS[1], QCH], F32, tag="psy", name=f"ps2_{q}"
                        )
                        nc.tensor.matmul(
                            ps2[:], lhsT=w_sb[0][:], rhs=p1[:],
                            start=True, stop=True,
                        )
                        nc.scalar.activation(
                            h_sb[:, q * QCH : (q + 1) * QCH],
                            ps2[:],
                            AF.Relu,
                            bias=b_sb[0][:],
                            scale=1.0 / (ASCALE * XSCALE),
                        )
                    else:
                        nc.scalar.activation(
                            h_sb[:, q * QCH : (q + 1) * QCH],
                            acc[q][:c_out, :],
                            AF.Relu,
                            bias=b_sb[l][:],
                            scale=1.0 / ASCALE,
                        )
                if l == 2:
                    break

                # ---- project local Y_{l+1} rows + two pipelined AllGathers
                c_next = DIMS[l + 2]
                stat = stat_pair(l + 1, c_next)
                for h in range(NHALF):
                    y_sb = ypool.tile(
                        [P, NU // 2, c_next], F8, tag="y", name=f"y{l}_{h}"
                    )
                    for u8 in range(NU // 2):
                        u = h * 8 + u8
                        ps = psml.tile(
                            [P, c_next], F32, tag="psy", name=f"psy{l}_{u}"
                        )
                        nc.tensor.matmul(
                            ps[:],
                            lhsT=h_sb[:, u * P : (u + 1) * P],
                            rhs=w_sb[l + 1][:],
                            start=True,
                            stop=True,
                        )
                        nc.vector.tensor_copy(out=y_sb[:, u8, :], in_=ps[:])
                    ag_in = dram.tile(
                        [P, NU // 2, c_next], F8, name=f"agin{l}_{h}"
                    )
                    ag_out = dram.tile(
                        [NCORES, P, NU // 2, c_next], F8, name=f"agout{l}_{h}",
                        addr_space="Shared",
                    )
                    nc.scalar.dma_start(ag_in[:], y_sb[:])
                    nc.gpsimd.collective_compute(
                        "AllGather",
                        OP.bypass,
                        replica_groups=rg,
                        ins=[ag_in[:].opt()],
                        outs=[ag_out[:].opt()],
                    )
                    nc.scalar.dma_start(
                        stat[h][:], ag_out[:].rearrange("r p u c -> p r u c")
                    )

            # ---- mean pool over all nodes ----
            gp = smal.tile([DIMS[3], 1], F32, name="gpart")
            nc.vector.tensor_reduce(
                gp[:], h_sb[:], axis=mybir.AxisListType.X, op=OP.add
            )
            ar_in = dram.tile([DIMS[3], 1], F32, name="arin")
            ar_out = dram.tile([DIMS[3], 1], F32, name="arout", addr_space="Shared")
            nc.scalar.dma_start(ar_in[:], gp[:])
            nc.gpsimd.collective_compute(
                "AllReduce",
                OP.add,
                replica_groups=rg,
                ins=[ar_in[:].opt()],
                outs=[ar_out[:].opt()],
            )
            g_sb = smal.tile([DIMS[3], 1], F32, name="gsb")
            nc.scalar.dma_start(g_sb[:], ar_out[:])
            nc.any.tensor_scalar_mul(g_sb[:], g_sb[:], 1.0 / N)

            # ---- MLP head: h1 = elu(g @ Wh1 + bh1) ----
            ps1 = psmlp.tile([32, 1], F32, tag="mlp", name="ps1")
            nc.tensor.matmul(ps1[:], lhsT=wh1_sb[:], rhs=g_sb[:], start=True, stop=True)
            # elu(x) = relu(x) + exp(min(x, 0)) - 1
            tmin = smal.tile([32, 1], F32, name="tmin")
            nc.vector.tensor_scalar(tmin[:], ps1[:], bh1_sb[:], 0.0, OP.add, OP.min)
            e1 = smal.tile([32, 1], F32, name="e1")
            nc.scalar.activation(e1[:], tmin[:], AF.Exp)
            r1 = smal.tile([32, 1], F32, name="r1")
            nc.scalar.activation(r1[:], ps1[:], AF.Relu, bias=bh1_sb[:])
            h1 = smal.tile([32, 1], F32, name="h1")
            nc.vector.tensor_tensor(h1[:], e1[:], r1[:], OP.add)
            nc.vector.tensor_scalar_add(h1[:], h1[:], -1.0)

            # ---- logits = h1 @ Wh2 + bh2; probs = softmax(logits) ----
            ps2m = psmlp.tile([2, 1], F32, tag="mlp", name="ps2m")
            nc.tensor.matmul(ps2m[:], lhsT=wh2_sb[:], rhs=h1[:], start=True, stop=True)
            logit_sb = smal.tile([2, 1], F32, name="logitsb")
            nc.vector.tensor_scalar(logit_sb[:], ps2m[:], bh2_sb[:], None, OP.add)
            nc.scalar.dma_start(logits_o.ap(), logit_sb[:])

            e2 = smal.tile([2, 1], F32, name="e2")
            nc.scalar.activation(e2[:], logit_sb[:], AF.Exp)
            ones21 = smal.tile([2, 1], F32, name="ones21")
            nc.any.memset(ones21[:], 1.0)
            ones12 = smal.tile([1, 2], F32, name="ones12")
            nc.any.memset(ones12[:], 1.0)
            ps3 = psmlp.tile([1, 1], F32, tag="mlp", name="ps3")
            nc.tensor.matmul(ps3[:], lhsT=e2[:], rhs=ones21[:], start=True, stop=True)
            rsc = smal.tile([1, 1], F32, name="rsc")
            nc.vector.reciprocal(rsc[:], ps3[:])
            ps4 = psmlp.tile([2, 1], F32, tag="mlp", name="ps4")
            nc.tensor.matmul(ps4[:], lhsT=ones12[:], rhs=rsc[:], start=True, stop=True)
            probs_sb = smal.tile([2, 1], F32, name="probssb")
            nc.vector.tensor_tensor(probs_sb[:], e2[:], ps4[:], OP.mult)
            nc.scalar.dma_start(probs_o.ap(), probs_sb[:])

    nc.finalize()
    return nc


def _install_ntff_hook():
    """Register the axon NTFF profiling hook if the container's antenv stub
    lacks it (bass_utils imports antenv.axon_hooks when trace=True)."""
    import sys
    import types

    try:
        import antenv.axon_hooks  # noqa: F401
        return
    except ImportError:
        pass
    mod = types.ModuleType("antenv.axon_hooks")
    _h = [None]
    mod.set_axon_ntff_profile_hook = lambda h: _h.__setitem__(0, h)
    mod.get_axon_ntff_profile_hook = lambda: _h[0]
    sys.modules["antenv.axon_hooks"] = mod
    import antenv

    antenv.axon_hooks = mod
    try:
        from trn_agent_boot import trn_boot

        hook = trn_boot._ntff_profile_via_ctypes("/opt/axon/libaxon_pjrt.so")
        if hook is not None:
            mod.set_axon_ntff_profile_hook(hook)
    except Exception:
        pass


def _get_nc():
    global _nc_cache
    if _nc_cache is None:
        _nc_cache = _build_nc()
    return _nc_cache


_last_results = None


def kernel(
    node_feat,
    adj_matrix,
    W0,
    b0,
    W1,
    b1,
    W2,
    b2,
    Wh1,
    bh1,
    Wh2,
    bh2,
):
    global _last_results
    import os

    node_feat = np.ascontiguousarray(np.asarray(node_feat, dtype=np.float32))
    adj = np.asarray(adj_matrix, dtype=np.float32)

    # ---- host-side sharding / preprocessing ----
    deg = adj.sum(axis=1, dtype=np.float32) + 1.0
    dinv = (1.0 / np.sqrt(deg)).astype(np.float32)

    fp8 = ml_dtypes.float8_e4m3
    f32c = lambda a, shape=None: np.ascontiguousarray(
        np.asarray(a, dtype=np.float32).reshape(shape)
        if shape is not None
        else np.asarray(a, dtype=np.float32)
    )

    # X scaled to fp8, tiled [p, h, r, u8, c]: node j = r*2048+h*1024+u8*128+p
    x8 = (node_feat * np.float32(XSCALE)).astype(fp8)
    x8 = np.ascontiguousarray(
        x8.reshape(NCORES, NHALF, NU // 2, P, DIMS[0]).transpose(3, 1, 0, 2, 4)
    )

    common = {
        "x8": x8,
        "w0": f32c(W0),
        "b0": f32c(b0, (-1, 1)),
        "w1": f32c(W1),
        "b1": f32c(b1, (-1, 1)),
        "w2": f32c(W2),
        "b2": f32c(b2, (-1, 1)),
        "wh1": f32c(Wh1),
        "bh1": f32c(bh1, (-1, 1)),
        "wh2": f32c(Wh2),
        "bh2": f32c(bh2, (-1, 1)),
    }

    in_maps = []
    idx = np.arange(ROWS)
    sdinv = dinv * np.float32(ASCALE)
    for k in range(NCORES):
        sl = slice(k * ROWS, (k + 1) * ROWS)
        # rows of ASCALE*Ahat for this core's output nodes
        blk = adj[sl, :] * sdinv[sl, None]
        blk *= dinv[None, :]
        blk[idx, k * ROWS + idx] = sdinv[sl] * dinv[sl]  # + I self loops
        a_k = blk.T.astype(fp8)  # [N, ROWS] = scaled Ahat.T cols
        # pre-tile to device layout [h, r, p, t, i]:
        # row j = r*2048 + h*1024 + t*128 + p
        a_k = np.ascontiguousarray(
            a_k.reshape(NCORES, NHALF, NSTRIPE, P, ROWS).transpose(1, 0, 3, 2, 4)
        )
        m = {"a_t": a_k}
        m.update(common)
        in_maps.append(m)

    from concourse import bass_utils

    nc = _get_nc()
    trace = bool(int(os.environ.get("GCN_TRACE", "0")))
    if trace:
        _install_ntff_hook()
    res = bass_utils.run_bass_kernel_spmd(
        nc, in_maps, core_ids=list(range(NCORES)), trace=trace
    )
    _last_results = res

    out0 = res.results[0]
    logits = np.asarray(out0["logits"], dtype=np.float32).reshape(2)
    probs = np.asarray(out0["probs"], dtype=np.float32).reshape(2)
    return (logits, probs)


# revision 25
# speedup vs baseline: 2.1361x; 1.0590x over previous
"""Trainium2 Bass kernel for a 3-layer GCN + mean-pool + MLP + softmax.

Reference computation (N=16384 nodes, dense adjacency):
    Ahat = D^-1/2 (A + I) D^-1/2
    H0 = X;  H_{l+1} = relu(Ahat @ (H_l @ W_l) + b_l)   l = 0,1,2
    g = mean(H3, axis=0);  h1 = elu(g @ Wh1 + bh1)
    logits = h1 @ Wh2 + bh2;  probs = softmax(logits)

Distribution (8 NeuronCores, 1D node/row parallel):
  - Host folds the symmetric degree normalization into the adjacency and
    ships each core the *transposed* normalized adjacency columns for its
    2048 output nodes as fp8 e4m3 (32MB/core), pre-tiled to the SBUF
    layout [half, rank, partition, stripe, i] so every adjacency DMA
    reads per-partition-contiguous runs.  ASCALE/XSCALE keep fp8 values
    in normal range and are divided back out by the relu's scale.
  - On device, the big matmul per layer streams the adjacency through the
    tensor engine (moving operand, DoubleRow fp8: 256-deep contraction)
    against stationary Y_l = H_l @ W_l tiles:
        out.T[c, i] = sum_j Y_l[j, c] * Ahat.T[j, i]   (PSUM fp32 accum)
  - Layer 1 uses associativity: Ahat @ (X W0) = (Ahat @ X) W0, with X
    itself (fp8, host-tiled) as the stationary — no device-side Y0 and no
    collective before layer 1, so the collective entry barrier and rank
    skew hide behind the adjacency stream.
  - Between layers: each core computes Y_{l+1} rows for its own nodes
    with a small fp32 matmul, then TWO half-node AllGathers replicate
    Y_{l+1}; the j-loop is ordered half-major so the second gather hides
    behind the first half's matmuls.  A tiny background AllReduce fires
    mid-layer so cross-core skew is absorbed on the CC cores instead of
    at the AllGather.
  - Mean pool: per-core partial sum over the free axis + AllReduce, then a
    replicated tiny MLP + softmax; core 0's output is returned.
  - DMA ring split: the bulk adjacency stream runs on the SP (nc.sync)
    HWDGE ring; all small loads that may wait on collectives run on the
    ACT (nc.scalar) ring so they never stall the adjacency stream.
"""

import numpy as np
import ml_dtypes

N = 16384
NCORES = 8
ROWS = N // NCORES          # 2048 output nodes per core
P = 128
DIMS = [64, 32, 48, 64]     # feature dims: in, after l0, l1, l2
NSTRIPE = 8                 # 128-row j-stripes per DMA group (2MB fp8)
NHALF = 2                   # half-node split for pipelined AllGathers
NGROUPS = NCORES * NHALF    # 16 groups per layer: (h, r)
QCH = 512                   # moving-operand free-dim chunk (1 PSUM bank)
NQ = ROWS // QCH            # 4
NU = ROWS // P              # 16 local node tiles
NDT = NSTRIPE // 2          # 4 double j-tiles per group (DoubleRow)
ASCALE = 16.0               # fp8 range helper for Ahat
XSCALE = 16.0               # fp8 range helper for X
ABUFS = 4                   # streamed adjacency groups in flight (8MB)
NCACHE = 6                  # adjacency groups cached in SBUF across layers

_nc_cache = None


def _build_nc():
    from concourse import bacc, mybir, tile

    dt = mybir.dt
    F32 = dt.float32
    F8 = dt.float8e4
    AF = mybir.ActivationFunctionType
    OP = mybir.AluOpType
    DR = mybir.MatmulPerfMode.DoubleRow

    nc = bacc.Bacc(
        "TRN2", target_bir_lowering=False, debug=False, num_devices=NCORES
    )

    # adjacency pre-tiled on host: [h, r, p, t, i]
    a_t = nc.dram_tensor(
        "a_t", [NHALF, NCORES, P, NSTRIPE, ROWS], F8, kind="ExternalInput"
    )
    # full X (scaled, fp8), pre-tiled partition-major so each partition's
    # stationary data is one contiguous run: [p, h, r, u8, c]
    x8 = nc.dram_tensor(
        "x8", [P, NHALF, NCORES, NU // 2, DIMS[0]], F8, kind="ExternalInput"
    )
    w_d = [
        nc.dram_tensor(f"w{l}", [DIMS[l], DIMS[l + 1]], F32, kind="ExternalInput")
        for l in range(3)
    ]
    b_d = [
        nc.dram_tensor(f"b{l}", [DIMS[l + 1], 1], F32, kind="ExternalInput")
        for l in range(3)
    ]
    wh1_d = nc.dram_tensor("wh1", [DIMS[3], 32], F32, kind="ExternalInput")
    bh1_d = nc.dram_tensor("bh1", [32, 1], F32, kind="ExternalInput")
    wh2_d = nc.dram_tensor("wh2", [32, 2], F32, kind="ExternalInput")
    bh2_d = nc.dram_tensor("bh2", [2, 1], F32, kind="ExternalInput")
    logits_o = nc.dram_tensor("logits", [2, 1], F32, kind="ExternalOutput")
    probs_o = nc.dram_tensor("probs", [2, 1], F32, kind="ExternalOutput")

    rg = [list(range(NCORES))]

    with tile.TileContext(nc) as tc:
        with (
            tc.tile_pool(name="const", bufs=1) as const,
            tc.tile_pool(name="apool", bufs=ABUFS) as apool,
            tc.tile_pool(name="cpool", bufs=NCACHE) as cpool,
            tc.tile_pool(name="spool", bufs=2) as spool,
            tc.tile_pool(name="hpool", bufs=2) as hpool,
            tc.tile_pool(name="ypool", bufs=2) as ypool,
            tc.tile_pool(name="smal", bufs=1) as smal,
            tc.tile_pool(name="accp", bufs=1, space="PSUM") as accp,
            tc.tile_pool(name="psml", bufs=3, space="PSUM") as psml,
            tc.tile_pool(name="psmlp", bufs=1, space="PSUM") as psmlp,
            tc.tile_pool(name="dram", bufs=1, space="DRAM") as dram,
        ):
            # ---- layer-1 stationary = X itself (fp8, host-tiled),
            #      loaded first so the tensor engine can start ASAP ----
            def stat_pair(l, c_out):
                return [
                    spool.tile(
                        [P, NCORES, NU // 2, c_out], F8,
                        tag=f"stat{h}", name=f"stat{l}_{h}",
                    )
                    for h in range(NHALF)
                ]

            stat = stat_pair(0, DIMS[0])
            for h in range(NHALF):
                nc.scalar.dma_start(stat[h][:], x8.ap()[:, h])

            # ---- PE pre-warm: dummy matmuls during the DMA ramp flip the
            #      HAM clock gate to 2.4 GHz before the real work arrives
            dm_w = smal.tile([P, 8], F8, name="dmw")
            nc.vector.memset(dm_w[:], 0.0)
            dm_x = smal.tile([P, QCH], F8, name="dmx")
            nc.vector.memset(dm_x[:], 0.0)
            for i in range(48):
                dps = psmlp.tile([8, QCH], F32, tag="mlp", name=f"dps{i}")
                nc.tensor.matmul(
                    dps[:], lhsT=dm_w[:], rhs=dm_x[:], start=True, stop=True
                )

            # ---- constants into SBUF (ACT ring — keep SP ring for A) ----
            def load(handle, shape, name, dtype=F32):
                t = const.tile(shape, dtype, name=name)
                nc.scalar.dma_start(t[:], handle.ap())
                return t

            w_sb = [
                load(w_d[l], [DIMS[l], DIMS[l + 1]], f"w{l}sb") for l in range(3)
            ]
            b_sb = [load(b_d[l], [DIMS[l + 1], 1], f"b{l}sb") for l in range(3)]
            wh1_sb = load(wh1_d, [DIMS[3], 32], "wh1sb")
            bh1_sb = load(bh1_d, [32, 1], "bh1sb")
            wh2_sb = load(wh2_d, [32, 2], "wh2sb")
            bh2_sb = load(bh2_d, [2, 1], "bh2sb")


            def resync(tag, dep_ap):
                # chain the trigger on `dep_ap` (a mid-layer adjacency tile)
                # so every rank fires this at the same point in its layer
                rs_src = smal.tile([1, 1], F32, name=f"rss_{tag}")
                nc.vector.tensor_copy(out=rs_src[:], in_=dep_ap)
                rs_in = dram.tile([1, 1], F32, name=f"rsin_{tag}")
                nc.scalar.dma_start(rs_in[:], rs_src[:])
                rs_out = dram.tile([1, 1], F32, name=f"rsout_{tag}")
                nc.gpsimd.collective_compute(
                    "AllReduce",
                    OP.add,
                    replica_groups=rg,
                    ins=[rs_in[:].opt()],
                    outs=[rs_out[:].opt()],
                )

            h_sb = None
            a_cached = {}
            for l in range(3):
                c_stat = DIMS[0] if l == 0 else DIMS[l + 1]
                c_out = DIMS[l + 1]
                acc = [
                    accp.tile([P, QCH], F32, tag=f"acc{q}", name=f"acc{l}_{q}")
                    for q in range(NQ)
                ]
                gi = 0
                for h in range(NHALF):
                    for r in range(NCORES):
                        if h == 0 and r < NCACHE:
                            if l == 0:
                                a_sb = cpool.tile(
                                    [P, NSTRIPE, ROWS], F8, tag="ac",
                                    name=f"ac{r}",
                                )
                                nc.sync.dma_start(a_sb[:], a_t.ap()[h, r])
                                a_cached[r] = a_sb
                            else:
                                a_sb = a_cached[r]
                        else:
                            a_sb = apool.tile(
                                [P, NSTRIPE, ROWS], F8, tag="a", name=f"a{l}_{h}_{r}"
                            )
                            nc.sync.dma_start(a_sb[:], a_t.ap()[h, r])
                        for t2 in range(NDT):
                            lw = stat[h][:, r, 2 * t2 : 2 * t2 + 2, :]
                            for q in range(NQ):
                                nc.tensor.matmul(
                                    acc[q][:c_stat, :],
                                    lhsT=lw,
                                    rhs=a_sb[
                                        :, 2 * t2 : 2 * t2 + 2,
                                        q * QCH : (q + 1) * QCH,
                                    ],
                                    start=(gi == 0 and t2 == 0),
                                    stop=(gi == NGROUPS - 1 and t2 == NDT - 1),
                                    perf_mode=DR,
                                )
                        gi += 1
                        if h == 1 and r == 4:
                            resync(f"rs{l}", a_sb[0:1, 0, 0:1])

                h_sb = hpool.tile([c_out, ROWS], F32, tag="h", name=f"h{l}")
                for q in range(NQ):
                    if l == 0:
                        # H1 chunk = relu((Ahat@X)chunk @ W0 / s + b0)
                        p1 = ypool.tile(
                            [DIMS[0], QCH], F32, tag="p1", name=f"p1_{q}"
                        )
                        nc.vector.tensor_copy(out=p1[:], in_=acc[q][: DIMS[0], :])
                        ps2 = psml.tile(
                            [DIMS[1], QCH], F32, tag="psy", name=f"ps2_{q}"
                        )
                        nc.tensor.matmul(
                            ps2[:], lhsT=w_sb[0][:], rhs=p1[:],
                            start=True, stop=True,
                        )
                        nc.scalar.activation(
                            h_sb[:, q * QCH : (q + 1) * QCH],
                            ps2[:],
                            AF.Relu,
                            bias=b_sb[0][:],
                            scale=1.0 / (ASCALE * XSCALE),
                        )
                    else:
                        nc.scalar.activation(
                            h_sb[:, q * QCH : (q + 1) * QCH],
                            acc[q][:c_out, :],
                            AF.Relu,
                            bias=b_sb[l][:],
                            scale=1.0 / ASCALE,
                        )
                if l == 2:
                    break

                # ---- project local Y_{l+1} rows + two pipelined AllGathers
                c_next = DIMS[l + 2]
                stat = stat_pair(l + 1, c_next)
                for h in range(NHALF):
                    y_sb = ypool.tile(
                        [P, NU // 2, c_next], F8, tag="y", name=f"y{l}_{h}"
                    )
                    for u8 in range(NU // 2):
                        u = h * 8 + u8
                        ps = psml.tile(
                            [P, c_next], F32, tag="psy", name=f"psy{l}_{u}"
                        )
                        nc.tensor.matmul(
                            ps[:],
                            lhsT=h_sb[:, u * P : (u + 1) * P],
                            rhs=w_sb[l + 1][:],
                            start=True,
                            stop=True,
                        )
                        nc.vector.tensor_copy(out=y_sb[:, u8, :], in_=ps[:])
                    ag_in = dram.tile(
                        [P, NU // 2, c_next], F8, name=f"agin{l}_{h}"
                    )
                    ag_out = dram.tile(
                        [NCORES, P, NU // 2, c_next], F8, name=f"agout{l}_{h}",
                        addr_space="Shared",
                    )
                    nc.scalar.dma_start(ag_in[:], y_sb[:])
                    nc.gpsimd.collective_compute(
                        "AllGather",
                        OP.bypass,
                        replica_groups=rg,
                        ins=[ag_in[:].opt()],
                        outs=[ag_out[:].opt()],
                    )
                    nc.scalar.dma_start(
                        stat[h][:], ag_out[:].rearrange("r p u c -> p r u c")
                    )

            # ---- mean pool over all nodes ----
            gp = smal.tile([DIMS[3], 1], F32, name="gpart")
            nc.vector.tensor_reduce(
                gp[:], h_sb[:], axis=mybir.AxisListType.X, op=OP.add
            )
            ar_in = dram.tile([DIMS[3], 1], F32, name="arin")
            ar_out = dram.tile([DIMS[3], 1], F32, name="arout", addr_space="Shared")
            nc.scalar.dma_start(ar_in[:], gp[:])
            nc.gpsimd.collective_compute(
                "AllReduce",
                OP.add,
                replica_groups=rg,
                ins=[ar_in[:].opt()],
                outs=[ar_out[:].opt()],
            )
            g_sb = smal.tile([DIMS[3], 1], F32, name="gsb")
            nc.scalar.dma_start(g_sb[:], ar_out[:])
            nc.any.tensor_scalar_mul(g_sb[:], g_sb[:], 1.0 / N)

            # ---- MLP head: h1 = elu(g @ Wh1 + bh1) ----
            ps1 = psmlp.tile([32, 1], F32, tag="mlp", name="ps1")
            nc.tensor.matmul(ps1[:], lhsT=wh1_sb[:], rhs=g_sb[:], start=True, stop=True)
            # elu(x) = relu(x) + exp(min(x, 0)) - 1
            tmin = smal.tile([32, 1], F32, name="tmin")
            nc.vector.tensor_scalar(tmin[:], ps1[:], bh1_sb[:], 0.0, OP.add, OP.min)
            e1 = smal.tile([32, 1], F32, name="e1")
            nc.scalar.activation(e1[:], tmin[:], AF.Exp)
            r1 = smal.tile([32, 1], F32, name="r1")
            nc.scalar.activation(r1[:], ps1[:], AF.Relu, bias=bh1_sb[:])
            h1 = smal.tile([32, 1], F32, name="h1")
            nc.vector.tensor_tensor(h1[:], e1[:], r1[:], OP.add)
            nc.vector.tensor_scalar_add(h1[:], h1[:], -1.0)

            # ---- logits = h1 @ Wh2 + bh2; probs = softmax(logits) ----
            ps2m = psmlp.tile([2, 1], F32, tag="mlp", name="ps2m")
            nc.tensor.matmul(ps2m[:], lhsT=wh2_sb[:], rhs=h1[:], start=True, stop=True)
            logit_sb = smal.tile([2, 1], F32, name="logitsb")
            nc.vector.tensor_scalar(logit_sb[:], ps2m[:], bh2_sb[:], None, OP.add)
            nc.scalar.dma_start(logits_o.ap(), logit_sb[:])

            e2 = smal.tile([2, 1], F32, name="e2")
            nc.scalar.activation(e2[:], ps2m[:], AF.Exp, bias=bh2_sb[:])
            ones21 = smal.tile([2, 1], F32, name="ones21")
            nc.any.memset(ones21[:], 1.0)
            ones12 = smal.tile([1, 2], F32, name="ones12")
            nc.any.memset(ones12[:], 1.0)
            ps3 = psmlp.tile([1, 1], F32, tag="mlp", name="ps3")
            nc.tensor.matmul(ps3[:], lhsT=e2[:], rhs=ones21[:], start=True, stop=True)
            rsc = smal.tile([1, 1], F32, name="rsc")
            nc.vector.reciprocal(rsc[:], ps3[:])
            ps4 = psmlp.tile([2, 1], F32, tag="mlp", name="ps4")
            nc.tensor.matmul(ps4[:], lhsT=ones12[:], rhs=rsc[:], start=True, stop=True)
            probs_sb = smal.tile([2, 1], F32, name="probssb")
            nc.vector.tensor_tensor(probs_sb[:], e2[:], ps4[:], OP.mult)
            nc.scalar.dma_start(probs_o.ap(), probs_sb[:])

    nc.finalize()
    return nc


def _install_ntff_hook():
    """Register the axon NTFF profiling hook if the container's antenv stub
    lacks it (bass_utils imports antenv.axon_hooks when trace=True)."""
    import sys
    import types

    try:
        import antenv.axon_hooks  # noqa: F401
        return
    except ImportError:
        pass
    mod = types.ModuleType("antenv.axon_hooks")
    _h = [None]
    mod.set_axon_ntff_profile_hook = lambda h: _h.__setitem__(0, h)
    mod.get_axon_ntff_profile_hook = lambda: _h[0]
    sys.modules["antenv.axon_hooks"] = mod
    import antenv

    antenv.axon_hooks = mod
    try:
        from trn_agent_boot import trn_boot

        hook = trn_boot._ntff_profile_via_ctypes("/opt/axon/libaxon_pjrt.so")
        if hook is not None:
            mod.set_axon_ntff_profile_hook(hook)
    except Exception:
        pass


def _get_nc():
    global _nc_cache
    if _nc_cache is None:
        _nc_cache = _build_nc()
    return _nc_cache


_last_results = None


def kernel(
    node_feat,
    adj_matrix,
    W0,
    b0,
    W1,
    b1,
    W2,
    b2,
    Wh1,
    bh1,
    Wh2,
    bh2,
):
    global _last_results
    import os

    node_feat = np.ascontiguousarray(np.asarray(node_feat, dtype=np.float32))
    adj = np.asarray(adj_matrix, dtype=np.float32)

    # ---- host-side sharding / preprocessing ----
    deg = adj.sum(axis=1, dtype=np.float32) + 1.0
    dinv = (1.0 / np.sqrt(deg)).astype(np.float32)

    fp8 = ml_dtypes.float8_e4m3
    f32c = lambda a, shape=None: np.ascontiguousarray(
        np.asarray(a, dtype=np.float32).reshape(shape)
        if shape is not None
        else np.asarray(a, dtype=np.float32)
    )

    # X scaled to fp8, tiled [p, h, r, u8, c]: node j = r*2048+h*1024+u8*128+p
    x8 = (node_feat * np.float32(XSCALE)).astype(fp8)
    x8 = np.ascontiguousarray(
        x8.reshape(NCORES, NHALF, NU // 2, P, DIMS[0]).transpose(3, 1, 0, 2, 4)
    )

    common = {
        "x8": x8,
        "w0": f32c(W0),
        "b0": f32c(b0, (-1, 1)),
        "w1": f32c(W1),
        "b1": f32c(b1, (-1, 1)),
        "w2": f32c(W2),
        "b2": f32c(b2, (-1, 1)),
        "wh1": f32c(Wh1),
        "bh1": f32c(bh1, (-1, 1)),
        "wh2": f32c(Wh2),
        "bh2": f32c(bh2, (-1, 1)),
    }

    in_maps = []
    idx = np.arange(ROWS)
    sdinv = dinv * np.float32(ASCALE)
    for k in range(NCORES):
        sl = slice(k * ROWS, (k + 1) * ROWS)
        # rows of ASCALE*Ahat for this core's output nodes
        blk = adj[sl, :] * sdinv[sl, None]
        blk *= dinv[None, :]
        blk[idx, k * ROWS + idx] = sdinv[sl] * dinv[sl]  # + I self loops
        a_k = blk.T.astype(fp8)  # [N, ROWS] = scaled Ahat.T cols
        # pre-tile to device layout [h, r, p, t, i]:
        # row j = r*2048 + h*1024 + t*128 + p
        a_k = np.ascontiguousarray(
            a_k.reshape(NCORES, NHALF, NSTRIPE, P, ROWS).transpose(1, 0, 3, 2, 4)
        )
        m = {"a_t": a_k}
        m.update(common)
        in_maps.append(m)

    from concourse import bass_utils

    nc = _get_nc()
    trace = bool(int(os.environ.get("GCN_TRACE", "0")))
    if trace:
        _install_ntff_hook()
    res = bass_utils.run_bass_kernel_spmd(
        nc, in_maps, core_ids=list(range(NCORES)), trace=trace
    )
    _last_results = res

    out0 = res.results[0]
    logits = np.asarray(out0["logits"], dtype=np.float32).reshape(2)
    probs = np.asarray(out0["probs"], dtype=np.float32).reshape(2)
    return (logits, probs)


# revision 26
# speedup vs baseline: 2.1733x; 1.0174x over previous
"""Trainium2 Bass kernel for a 3-layer GCN + mean-pool + MLP + softmax.

Reference computation (N=16384 nodes, dense adjacency):
    Ahat = D^-1/2 (A + I) D^-1/2
    H0 = X;  H_{l+1} = relu(Ahat @ (H_l @ W_l) + b_l)   l = 0,1,2
    g = mean(H3, axis=0);  h1 = elu(g @ Wh1 + bh1)
    logits = h1 @ Wh2 + bh2;  probs = softmax(logits)

Distribution (8 NeuronCores, 1D node/row parallel):
  - Host folds the symmetric degree normalization into the adjacency and
    ships each core the *transposed* normalized adjacency columns for its
    2048 output nodes as fp8 e4m3 (32MB/core), pre-tiled to the SBUF
    layout [half, rank, partition, stripe, i] so every adjacency DMA
    reads per-partition-contiguous runs.  ASCALE/XSCALE keep fp8 values
    in normal range and are divided back out by the relu's scale.
  - On device, the big matmul per layer streams the adjacency through the
    tensor engine (moving operand, DoubleRow fp8: 256-deep contraction)
    against stationary Y_l = H_l @ W_l tiles:
        out.T[c, i] = sum_j Y_l[j, c] * Ahat.T[j, i]   (PSUM fp32 accum)
  - Layer 1 uses associativity: Ahat @ (X W0) = (Ahat @ X) W0, with X
    itself (fp8, host-tiled) as the stationary — no device-side Y0 and no
    collective before layer 1, so the collective entry barrier and rank
    skew hide behind the adjacency stream.
  - Between layers: each core computes Y_{l+1} rows for its own nodes
    with a small fp32 matmul, then TWO half-node AllGathers replicate
    Y_{l+1}; the j-loop is ordered half-major so the second gather hides
    behind the first half's matmuls.  A tiny background AllReduce fires
    mid-layer so cross-core skew is absorbed on the CC cores instead of
    at the AllGather.
  - Mean pool: per-core partial sum over the free axis + AllReduce, then a
    replicated tiny MLP + softmax; core 0's output is returned.
  - DMA ring split: the bulk adjacency stream runs on the SP (nc.sync)
    HWDGE ring; all small loads that may wait on collectives run on the
    ACT (nc.scalar) ring so they never stall the adjacency stream.
"""

import numpy as np
import ml_dtypes

N = 16384
NCORES = 8
ROWS = N // NCORES          # 2048 output nodes per core
P = 128
DIMS = [64, 32, 48, 64]     # feature dims: in, after l0, l1, l2
NSTRIPE = 8                 # 128-row j-stripes per DMA group (2MB fp8)
NHALF = 2                   # half-node split for pipelined AllGathers
NGROUPS = NCORES * NHALF    # 16 groups per layer: (h, r)
QCH = 512                   # moving-operand free-dim chunk (1 PSUM bank)
NQ = ROWS // QCH            # 4
NU = ROWS // P              # 16 local node tiles
NDT = NSTRIPE // 2          # 4 double j-tiles per group (DoubleRow)
ASCALE = 16.0               # fp8 range helper for Ahat
XSCALE = 16.0               # fp8 range helper for X
ABUFS = 6                   # streamed adjacency groups in flight (12MB)
NCACHE = 5                  # adjacency groups cached in SBUF across layers

_nc_cache = None


def _build_nc():
    from concourse import bacc, mybir, tile

    dt = mybir.dt
    F32 = dt.float32
    F8 = dt.float8e4
    BF16 = dt.bfloat16
    AF = mybir.ActivationFunctionType
    OP = mybir.AluOpType
    DR = mybir.MatmulPerfMode.DoubleRow

    nc = bacc.Bacc(
        "TRN2", target_bir_lowering=False, debug=False, num_devices=NCORES
    )

    # adjacency pre-tiled on host: [h, r, p, t, i]
    a_t = nc.dram_tensor(
        "a_t", [NHALF, NCORES, P, NSTRIPE, ROWS], F8, kind="ExternalInput"
    )
    # full X (scaled, fp8), pre-tiled partition-major so each partition's
    # stationary data is one contiguous run: [p, h, r, u8, c]
    x8 = nc.dram_tensor(
        "x8", [P, NHALF, NCORES, NU // 2, DIMS[0]], F8, kind="ExternalInput"
    )
    w_d = [
        nc.dram_tensor(
            f"w{l}", [DIMS[l], DIMS[l + 1]], F32 if l == 0 else BF16,
            kind="ExternalInput",
        )
        for l in range(3)
    ]
    b_d = [
        nc.dram_tensor(f"b{l}", [DIMS[l + 1], 1], F32, kind="ExternalInput")
        for l in range(3)
    ]
    wh1_d = nc.dram_tensor("wh1", [DIMS[3], 32], F32, kind="ExternalInput")
    bh1_d = nc.dram_tensor("bh1", [32, 1], F32, kind="ExternalInput")
    wh2_d = nc.dram_tensor("wh2", [32, 2], F32, kind="ExternalInput")
    bh2_d = nc.dram_tensor("bh2", [2, 1], F32, kind="ExternalInput")
    logits_o = nc.dram_tensor("logits", [2, 1], F32, kind="ExternalOutput")
    probs_o = nc.dram_tensor("probs", [2, 1], F32, kind="ExternalOutput")

    rg = [list(range(NCORES))]

    with tile.TileContext(nc) as tc:
        with (
            tc.tile_pool(name="const", bufs=1) as const,
            tc.tile_pool(name="apool", bufs=ABUFS) as apool,
            tc.tile_pool(name="cpool", bufs=NCACHE) as cpool,
            tc.tile_pool(name="spool", bufs=2) as spool,
            tc.tile_pool(name="hpool", bufs=2) as hpool,
            tc.tile_pool(name="ypool", bufs=2) as ypool,
            tc.tile_pool(name="smal", bufs=1) as smal,
            tc.tile_pool(name="accp", bufs=1, space="PSUM") as accp,
            tc.tile_pool(name="psml", bufs=3, space="PSUM") as psml,
            tc.tile_pool(name="psmlp", bufs=1, space="PSUM") as psmlp,
            tc.tile_pool(name="dram", bufs=1, space="DRAM") as dram,
        ):
            # ---- layer-1 stationary = X itself (fp8, host-tiled),
            #      loaded first so the tensor engine can start ASAP ----
            def stat_pair(l, c_out):
                return [
                    spool.tile(
                        [P, NCORES, NU // 2, c_out], F8,
                        tag=f"stat{h}", name=f"stat{l}_{h}",
                    )
                    for h in range(NHALF)
                ]

            stat = stat_pair(0, DIMS[0])
            for h in range(NHALF):
                nc.scalar.dma_start(stat[h][:], x8.ap()[:, h])

            # ---- PE pre-warm: dummy matmuls during the DMA ramp flip the
            #      HAM clock gate to 2.4 GHz before the real work arrives
            dm_w = smal.tile([P, 8], F8, name="dmw")
            nc.vector.memset(dm_w[:], 0.0)
            dm_x = smal.tile([P, QCH], F8, name="dmx")
            nc.vector.memset(dm_x[:], 0.0)
            for i in range(48):
                dps = psmlp.tile([8, QCH], F32, tag="mlp", name=f"dps{i}")
                nc.tensor.matmul(
                    dps[:], lhsT=dm_w[:], rhs=dm_x[:], start=True, stop=True
                )

            # ---- constants into SBUF (ACT ring — keep SP ring for A) ----
            def load(handle, shape, name, dtype=F32):
                t = const.tile(shape, dtype, name=name)
                nc.scalar.dma_start(t[:], handle.ap())
                return t

            w_sb = [
                load(
                    w_d[l], [DIMS[l], DIMS[l + 1]], f"w{l}sb",
                    dtype=F32 if l == 0 else BF16,
                )
                for l in range(3)
            ]
            b_sb = [load(b_d[l], [DIMS[l + 1], 1], f"b{l}sb") for l in range(3)]
            wh1_sb = load(wh1_d, [DIMS[3], 32], "wh1sb")
            bh1_sb = load(bh1_d, [32, 1], "bh1sb")
            wh2_sb = load(wh2_d, [32, 2], "wh2sb")
            bh2_sb = load(bh2_d, [2, 1], "bh2sb")


            def resync(tag, dep_ap):
                # chain the trigger on `dep_ap` (a mid-layer adjacency tile)
                # so every rank fires this at the same point in its layer
                rs_src = smal.tile([1, 1], F32, name=f"rss_{tag}")
                nc.vector.tensor_copy(out=rs_src[:], in_=dep_ap)
                rs_in = dram.tile([1, 1], F32, name=f"rsin_{tag}")
                nc.scalar.dma_start(rs_in[:], rs_src[:])
                rs_out = dram.tile([1, 1], F32, name=f"rsout_{tag}")
                nc.gpsimd.collective_compute(
                    "AllReduce",
                    OP.add,
                    replica_groups=rg,
                    ins=[rs_in[:].opt()],
                    outs=[rs_out[:].opt()],
                )

            h_sb = None
            a_cached = {}
            for l in range(3):
                c_stat = DIMS[0] if l == 0 else DIMS[l + 1]
                c_out = DIMS[l + 1]
                acc = [
                    accp.tile([P, QCH], F32, tag=f"acc{q}", name=f"acc{l}_{q}")
                    for q in range(NQ)
                ]
                gi = 0
                for h in range(NHALF):
                    for r in range(NCORES):
                        if h == 0 and r < NCACHE:
                            if l == 0:
                                a_sb = cpool.tile(
                                    [P, NSTRIPE, ROWS], F8, tag="ac",
                                    name=f"ac{r}",
                                )
                                nc.sync.dma_start(a_sb[:], a_t.ap()[h, r])
                                a_cached[r] = a_sb
                            else:
                                a_sb = a_cached[r]
                        else:
                            a_sb = apool.tile(
                                [P, NSTRIPE, ROWS], F8, tag="a", name=f"a{l}_{h}_{r}"
                            )
                            nc.sync.dma_start(a_sb[:], a_t.ap()[h, r])
                        for t2 in range(NDT):
                            lw = stat[h][:, r, 2 * t2 : 2 * t2 + 2, :]
                            for q in range(NQ):
                                nc.tensor.matmul(
                                    acc[q][:c_stat, :],
                                    lhsT=lw,
                                    rhs=a_sb[
                                        :, 2 * t2 : 2 * t2 + 2,
                                        q * QCH : (q + 1) * QCH,
                                    ],
                                    start=(gi == 0 and t2 == 0),
                                    stop=(gi == NGROUPS - 1 and t2 == NDT - 1),
                                    perf_mode=DR,
                                )
                        gi += 1
                        if h == 1 and r == 4:
                            resync(f"rs{l}", a_sb[0:1, 0, 0:1])

                h_sb = hpool.tile([c_out, ROWS], BF16, tag="h", name=f"h{l}")
                for q in range(NQ):
                    if l == 0:
                        # H1 chunk = relu((Ahat@X)chunk @ W0 / s + b0)
                        p1 = ypool.tile(
                            [DIMS[0], QCH], F32, tag="p1", name=f"p1_{q}"
                        )
                        nc.vector.tensor_copy(out=p1[:], in_=acc[q][: DIMS[0], :])
                        ps2 = psml.tile(
                            [DIMS[1], QCH], F32, tag="psy", name=f"ps2_{q}"
                        )
                        nc.tensor.matmul(
                            ps2[:], lhsT=w_sb[0][:], rhs=p1[:],
                            start=True, stop=True,
                        )
                        nc.scalar.activation(
                            h_sb[:, q * QCH : (q + 1) * QCH],
                            ps2[:],
                            AF.Relu,
                            bias=b_sb[0][:],
                            scale=1.0 / (ASCALE * XSCALE),
                        )
                    else:
                        nc.scalar.activation(
                            h_sb[:, q * QCH : (q + 1) * QCH],
                            acc[q][:c_out, :],
                            AF.Relu,
                            bias=b_sb[l][:],
                            scale=1.0 / ASCALE,
                        )
                if l == 2:
                    break

                # ---- project local Y_{l+1} rows + two pipelined AllGathers
                c_next = DIMS[l + 2]
                stat = stat_pair(l + 1, c_next)
                for h in range(NHALF):
                    y_sb = ypool.tile(
                        [P, NU // 2, c_next], F8, tag="y", name=f"y{l}_{h}"
                    )
                    for u8 in range(NU // 2):
                        u = h * 8 + u8
                        ps = psml.tile(
                            [P, c_next], F32, tag="psy", name=f"psy{l}_{u}"
                        )
                        nc.tensor.matmul(
                            ps[:],
                            lhsT=h_sb[:, u * P : (u + 1) * P],
                            rhs=w_sb[l + 1][:],
                            start=True,
                            stop=True,
                        )
                        nc.vector.tensor_copy(out=y_sb[:, u8, :], in_=ps[:])
                    ag_in = dram.tile(
                        [P, NU // 2, c_next], F8, name=f"agin{l}_{h}"
                    )
                    ag_out = dram.tile(
                        [NCORES, P, NU // 2, c_next], F8, name=f"agout{l}_{h}",
                        addr_space="Shared",
                    )
                    nc.scalar.dma_start(ag_in[:], y_sb[:])
                    nc.gpsimd.collective_compute(
                        "AllGather",
                        OP.bypass,
                        replica_groups=rg,
                        ins=[ag_in[:].opt()],
                        outs=[ag_out[:].opt()],
                    )
                    nc.scalar.dma_start(
                        stat[h][:], ag_out[:].rearrange("r p u c -> p r u c")
                    )

            # ---- mean pool over all nodes ----
            gp = smal.tile([DIMS[3], 1], F32, name="gpart")
            nc.vector.tensor_reduce(
                gp[:], h_sb[:], axis=mybir.AxisListType.X, op=OP.add
            )
            ar_in = dram.tile([DIMS[3], 1], F32, name="arin")
            ar_out = dram.tile([DIMS[3], 1], F32, name="arout", addr_space="Shared")
            nc.scalar.dma_start(ar_in[:], gp[:])
            nc.gpsimd.collective_compute(
                "AllReduce",
                OP.add,
                replica_groups=rg,
                ins=[ar_in[:].opt()],
                outs=[ar_out[:].opt()],
            )
            g_sb = smal.tile([DIMS[3], 1], F32, name="gsb")
            nc.scalar.dma_start(g_sb[:], ar_out[:])
            nc.any.tensor_scalar_mul(g_sb[:], g_sb[:], 1.0 / N)

            # ---- MLP head: h1 = elu(g @ Wh1 + bh1) ----
            ps1 = psmlp.tile([32, 1], F32, tag="mlp", name="ps1")
            nc.tensor.matmul(ps1[:], lhsT=wh1_sb[:], rhs=g_sb[:], start=True, stop=True)
            # elu(x) = relu(x) + exp(min(x, 0)) - 1
            tmin = smal.tile([32, 1], F32, name="tmin")
            nc.vector.tensor_scalar(tmin[:], ps1[:], bh1_sb[:], 0.0, OP.add, OP.min)
            e1 = smal.tile([32, 1], F32, name="e1")
            nc.scalar.activation(e1[:], tmin[:], AF.Exp)
            r1 = smal.tile([32, 1], F32, name="r1")
            nc.scalar.activation(r1[:], ps1[:], AF.Relu, bias=bh1_sb[:])
            h1 = smal.tile([32, 1], F32, name="h1")
            nc.vector.tensor_tensor(h1[:], e1[:], r1[:], OP.add)
            nc.vector.tensor_scalar_add(h1[:], h1[:], -1.0)

            # ---- logits = h1 @ Wh2 + bh2; probs = softmax(logits) ----
            ps2m = psmlp.tile([2, 1], F32, tag="mlp", name="ps2m")
            nc.tensor.matmul(ps2m[:], lhsT=wh2_sb[:], rhs=h1[:], start=True, stop=True)
            logit_sb = smal.tile([2, 1], F32, name="logitsb")
            nc.vector.tensor_scalar(logit_sb[:], ps2m[:], bh2_sb[:], None, OP.add)
            nc.scalar.dma_start(logits_o.ap(), logit_sb[:])

            e2 = smal.tile([2, 1], F32, name="e2")
            nc.scalar.activation(e2[:], ps2m[:], AF.Exp, bias=bh2_sb[:])
            ones21 = smal.tile([2, 1], F32, name="ones21")
            nc.any.memset(ones21[:], 1.0)
            ones12 = smal.tile([1, 2], F32, name="ones12")
            nc.any.memset(ones12[:], 1.0)
            ps3 = psmlp.tile([1, 1], F32, tag="mlp", name="ps3")
            nc.tensor.matmul(ps3[:], lhsT=e2[:], rhs=ones21[:], start=True, stop=True)
            rsc = smal.tile([1, 1], F32, name="rsc")
            nc.vector.reciprocal(rsc[:], ps3[:])
            ps4 = psmlp.tile([2, 1], F32, tag="mlp", name="ps4")
            nc.tensor.matmul(ps4[:], lhsT=ones12[:], rhs=rsc[:], start=True, stop=True)
            probs_sb = smal.tile([2, 1], F32, name="probssb")
            nc.vector.tensor_tensor(probs_sb[:], e2[:], ps4[:], OP.mult)
            nc.scalar.dma_start(probs_o.ap(), probs_sb[:])

    nc.finalize()
    return nc


def _install_ntff_hook():
    """Register the axon NTFF profiling hook if the container's antenv stub
    lacks it (bass_utils imports antenv.axon_hooks when trace=True)."""
    import sys
    import types

    try:
        import antenv.axon_hooks  # noqa: F401
        return
    except ImportError:
        pass
    mod = types.ModuleType("antenv.axon_hooks")
    _h = [None]
    mod.set_axon_ntff_profile_hook = lambda h: _h.__setitem__(0, h)
    mod.get_axon_ntff_profile_hook = lambda: _h[0]
    sys.modules["antenv.axon_hooks"] = mod
    import antenv

    antenv.axon_hooks = mod
    try:
        from trn_agent_boot import trn_boot

        hook = trn_boot._ntff_profile_via_ctypes("/opt/axon/libaxon_pjrt.so")
        if hook is not None:
            mod.set_axon_ntff_profile_hook(hook)
    except Exception:
        pass


def _get_nc():
    global _nc_cache
    if _nc_cache is None:
        _nc_cache = _build_nc()
    return _nc_cache


_last_results = None


def kernel(
    node_feat,
    adj_matrix,
    W0,
    b0,
    W1,
    b1,
    W2,
    b2,
    Wh1,
    bh1,
    Wh2,
    bh2,
):
    global _last_results
    import os

    node_feat = np.ascontiguousarray(np.asarray(node_feat, dtype=np.float32))
    adj = np.asarray(adj_matrix, dtype=np.float32)

    # ---- host-side sharding / preprocessing ----
    deg = adj.sum(axis=1, dtype=np.float32) + 1.0
    dinv = (1.0 / np.sqrt(deg)).astype(np.float32)

    fp8 = ml_dtypes.float8_e4m3
    bf16 = ml_dtypes.bfloat16
    f32c = lambda a, shape=None: np.ascontiguousarray(
        np.asarray(a, dtype=np.float32).reshape(shape)
        if shape is not None
        else np.asarray(a, dtype=np.float32)
    )

    # X scaled to fp8, tiled [p, h, r, u8, c]: node j = r*2048+h*1024+u8*128+p
    x8 = (node_feat * np.float32(XSCALE)).astype(fp8)
    x8 = np.ascontiguousarray(
        x8.reshape(NCORES, NHALF, NU // 2, P, DIMS[0]).transpose(3, 1, 0, 2, 4)
    )

    common = {
        "x8": x8,
        "w0": f32c(W0),
        "b0": f32c(b0, (-1, 1)),
        "w1": np.ascontiguousarray(np.asarray(W1, np.float32)).astype(bf16),
        "b1": f32c(b1, (-1, 1)),
        "w2": np.ascontiguousarray(np.asarray(W2, np.float32)).astype(bf16),
        "b2": f32c(b2, (-1, 1)),
        "wh1": f32c(Wh1),
        "bh1": f32c(bh1, (-1, 1)),
        "wh2": f32c(Wh2),
        "bh2": f32c(bh2, (-1, 1)),
    }

    in_maps = []
    idx = np.arange(ROWS)
    sdinv = dinv * np.float32(ASCALE)
    for k in range(NCORES):
        sl = slice(k * ROWS, (k + 1) * ROWS)
        # rows of ASCALE*Ahat for this core's output nodes
        blk = adj[sl, :] * sdinv[sl, None]
        blk *= dinv[None, :]
        blk[idx, k * ROWS + idx] = sdinv[sl] * dinv[sl]  # + I self loops
        a_k = blk.T.astype(fp8)  # [N, ROWS] = scaled Ahat.T cols
        # pre-tile to device layout [h, r, p, t, i]:
        # row j = r*2048 + h*1024 + t*128 + p
        a_k = np.ascontiguousarray(
            a_k.reshape(NCORES, NHALF, NSTRIPE, P, ROWS).transpose(1, 0, 3, 2, 4)
        )
        m = {"a_t": a_k}
        m.update(common)
        in_maps.append(m)

    from concourse import bass_utils

    nc = _get_nc()
    trace = bool(int(os.environ.get("GCN_TRACE", "0")))
    if trace:
        _install_ntff_hook()
    res = bass_utils.run_bass_kernel_spmd(
        nc, in_maps, core_ids=list(range(NCORES)), trace=trace
    )
    _last_results = res

    out0 = res.results[0]
    logits = np.asarray(out0["logits"], dtype=np.float32).reshape(2)
    probs = np.asarray(out0["probs"], dtype=np.float32).reshape(2)
    return (logits, probs)


# revision 27
# speedup vs baseline: 2.2124x; 1.0180x over previous
"""Trainium2 Bass kernel for a 3-layer GCN + mean-pool + MLP + softmax.

Reference computation (N=16384 nodes, dense adjacency):
    Ahat = D^-1/2 (A + I) D^-1/2
    H0 = X;  H_{l+1} = relu(Ahat @ (H_l @ W_l) + b_l)   l = 0,1,2
    g = mean(H3, axis=0);  h1 = elu(g @ Wh1 + bh1)
    logits = h1 @ Wh2 + bh2;  probs = softmax(logits)

Distribution (8 NeuronCores, 1D node/row parallel):
  - Host folds the symmetric degree normalization into the adjacency and
    ships each core the *transposed* normalized adjacency columns for its
    2048 output nodes as fp8 e4m3 (32MB/core), pre-tiled to the SBUF
    layout [half, rank, partition, stripe, i] so every adjacency DMA
    reads per-partition-contiguous runs.  ASCALE/XSCALE keep fp8 values
    in normal range and are divided back out by the relu's scale.
  - On device, the big matmul per layer streams the adjacency through the
    tensor engine (moving operand, DoubleRow fp8: 256-deep contraction)
    against stationary Y_l = H_l @ W_l tiles:
        out.T[c, i] = sum_j Y_l[j, c] * Ahat.T[j, i]   (PSUM fp32 accum)
  - Layer 1 uses associativity: Ahat @ (X W0) = (Ahat @ X) W0, with X
    itself (fp8, host-tiled) as the stationary — no device-side Y0 and no
    collective before layer 1, so the collective entry barrier and rank
    skew hide behind the adjacency stream.
  - Between layers: each core computes Y_{l+1} rows for its own nodes
    with a small fp32 matmul, then TWO half-node AllGathers replicate
    Y_{l+1}; the j-loop is ordered half-major so the second gather hides
    behind the first half's matmuls.  A tiny background AllReduce fires
    mid-layer so cross-core skew is absorbed on the CC cores instead of
    at the AllGather.
  - Mean pool: per-core partial sum over the free axis + AllReduce, then a
    replicated tiny MLP + softmax; core 0's output is returned.
  - DMA ring split: the bulk adjacency stream runs on the SP (nc.sync)
    HWDGE ring; all small loads that may wait on collectives run on the
    ACT (nc.scalar) ring so they never stall the adjacency stream.
"""

import numpy as np
import ml_dtypes

N = 16384
NCORES = 8
ROWS = N // NCORES          # 2048 output nodes per core
P = 128
DIMS = [64, 32, 48, 64]     # feature dims: in, after l0, l1, l2
NSTRIPE = 8                 # 128-row j-stripes per DMA group (2MB fp8)
NHALF = 2                   # half-node split for pipelined AllGathers
NGROUPS = NCORES * NHALF    # 16 groups per layer: (h, r)
QCH = 512                   # moving-operand free-dim chunk (1 PSUM bank)
NQ = ROWS // QCH            # 4
NU = ROWS // P              # 16 local node tiles
NDT = NSTRIPE // 2          # 4 double j-tiles per group (DoubleRow)
ASCALE = 16.0               # fp8 range helper for Ahat
XSCALE = 16.0               # fp8 range helper for X
ABUFS = 6                   # streamed adjacency groups in flight (12MB)
NCACHE = 5                  # adjacency groups cached in SBUF across layers

_nc_cache = None


def _build_nc():
    from concourse import bacc, mybir, tile

    dt = mybir.dt
    F32 = dt.float32
    F8 = dt.float8e4
    BF16 = dt.bfloat16
    AF = mybir.ActivationFunctionType
    OP = mybir.AluOpType
    DR = mybir.MatmulPerfMode.DoubleRow

    nc = bacc.Bacc(
        "TRN2", target_bir_lowering=False, debug=False, num_devices=NCORES
    )

    # adjacency pre-tiled on host: [h, r, p, t, i]
    a_t = nc.dram_tensor(
        "a_t", [NHALF, NCORES, P, NSTRIPE, ROWS], F8, kind="ExternalInput"
    )
    # full X (scaled, fp8), pre-tiled partition-major so each partition's
    # stationary data is one contiguous run: [p, h, r, u8, c]
    x8 = nc.dram_tensor(
        "x8", [P, NHALF, NCORES, NU // 2, DIMS[0]], F8, kind="ExternalInput"
    )
    w_d = [
        nc.dram_tensor(
            f"w{l}", [DIMS[l], DIMS[l + 1]], F32 if l == 0 else BF16,
            kind="ExternalInput",
        )
        for l in range(3)
    ]
    b_d = [
        nc.dram_tensor(f"b{l}", [DIMS[l + 1], 1], F32, kind="ExternalInput")
        for l in range(3)
    ]
    wh1_d = nc.dram_tensor("wh1", [DIMS[3], 32], F32, kind="ExternalInput")
    bh1_d = nc.dram_tensor("bh1", [32, 1], F32, kind="ExternalInput")
    wh2_d = nc.dram_tensor("wh2", [32, 2], F32, kind="ExternalInput")
    bh2_d = nc.dram_tensor("bh2", [2, 1], F32, kind="ExternalInput")
    logits_o = nc.dram_tensor("logits", [2, 1], F32, kind="ExternalOutput")
    probs_o = nc.dram_tensor("probs", [2, 1], F32, kind="ExternalOutput")

    rg = [list(range(NCORES))]

    with tile.TileContext(nc) as tc:
        with (
            tc.tile_pool(name="const", bufs=1) as const,
            tc.tile_pool(name="apool", bufs=ABUFS) as apool,
            tc.tile_pool(name="cpool", bufs=NCACHE) as cpool,
            tc.tile_pool(name="spool", bufs=2) as spool,
            tc.tile_pool(name="hpool", bufs=2) as hpool,
            tc.tile_pool(name="ypool", bufs=2) as ypool,
            tc.tile_pool(name="smal", bufs=1) as smal,
            tc.tile_pool(name="accp", bufs=1, space="PSUM") as accp,
            tc.tile_pool(name="psml", bufs=3, space="PSUM") as psml,
            tc.tile_pool(name="psmlp", bufs=1, space="PSUM") as psmlp,
            tc.tile_pool(name="dram", bufs=1, space="DRAM") as dram,
        ):
            # ---- layer-1 stationary = X itself (fp8, host-tiled),
            #      loaded first so the tensor engine can start ASAP ----
            def stat_pair(l, c_out):
                return [
                    spool.tile(
                        [P, NCORES, NU // 2, c_out], F8,
                        tag=f"stat{h}", name=f"stat{l}_{h}",
                    )
                    for h in range(NHALF)
                ]

            stat = stat_pair(0, DIMS[0])
            for h in range(NHALF):
                nc.scalar.dma_start(stat[h][:], x8.ap()[:, h])

            # ---- PE pre-warm: dummy matmuls during the DMA ramp flip the
            #      HAM clock gate to 2.4 GHz before the real work arrives
            dm_w = smal.tile([P, 8], F8, name="dmw")
            nc.vector.memset(dm_w[:], 0.0)
            dm_x = smal.tile([P, QCH], F8, name="dmx")
            nc.vector.memset(dm_x[:], 0.0)
            for i in range(48):
                dps = psmlp.tile([8, QCH], F32, tag="mlp", name=f"dps{i}")
                nc.tensor.matmul(
                    dps[:], lhsT=dm_w[:], rhs=dm_x[:], start=True, stop=True
                )

            # ---- constants into SBUF (ACT ring — keep SP ring for A) ----
            def load(handle, shape, name, dtype=F32):
                t = const.tile(shape, dtype, name=name)
                nc.scalar.dma_start(t[:], handle.ap())
                return t

            w_sb = [
                load(
                    w_d[l], [DIMS[l], DIMS[l + 1]], f"w{l}sb",
                    dtype=F32 if l == 0 else BF16,
                )
                for l in range(3)
            ]
            b_sb = [load(b_d[l], [DIMS[l + 1], 1], f"b{l}sb") for l in range(3)]
            wh1_sb = load(wh1_d, [DIMS[3], 32], "wh1sb")
            bh1_sb = load(bh1_d, [32, 1], "bh1sb")
            wh2_sb = load(wh2_d, [32, 2], "wh2sb")
            bh2_sb = load(bh2_d, [2, 1], "bh2sb")


            def resync(tag, dep_ap):
                # chain the trigger on `dep_ap` (a mid-layer adjacency tile)
                # so every rank fires this at the same point in its layer
                rs_src = smal.tile([1, 1], F32, name=f"rss_{tag}")
                nc.vector.tensor_copy(out=rs_src[:], in_=dep_ap)
                rs_in = dram.tile([1, 1], F32, name=f"rsin_{tag}")
                nc.scalar.dma_start(rs_in[:], rs_src[:])
                rs_out = dram.tile([1, 1], F32, name=f"rsout_{tag}")
                nc.gpsimd.collective_compute(
                    "AllReduce",
                    OP.add,
                    replica_groups=rg,
                    ins=[rs_in[:].opt()],
                    outs=[rs_out[:].opt()],
                )

            h_sb = None
            a_cached = {}
            for l in range(3):
                c_stat = DIMS[0] if l == 0 else DIMS[l + 1]
                c_out = DIMS[l + 1]
                acc = [
                    accp.tile([P, QCH], F32, tag=f"acc{q}", name=f"acc{l}_{q}")
                    for q in range(NQ)
                ]
                gi = 0
                for h in range(NHALF):
                    for r in range(NCORES):
                        if h == 0 and r < NCACHE:
                            if l == 0:
                                a_sb = cpool.tile(
                                    [P, NSTRIPE, ROWS], F8, tag="ac",
                                    name=f"ac{r}",
                                )
                                nc.sync.dma_start(a_sb[:], a_t.ap()[h, r])
                                a_cached[r] = a_sb
                            else:
                                a_sb = a_cached[r]
                        else:
                            a_sb = apool.tile(
                                [P, NSTRIPE, ROWS], F8, tag="a", name=f"a{l}_{h}_{r}"
                            )
                            nc.sync.dma_start(a_sb[:], a_t.ap()[h, r])
                        for t2 in range(NDT):
                            lw = stat[h][:, r, 2 * t2 : 2 * t2 + 2, :]
                            for q in range(NQ):
                                nc.tensor.matmul(
                                    acc[q][:c_stat, :],
                                    lhsT=lw,
                                    rhs=a_sb[
                                        :, 2 * t2 : 2 * t2 + 2,
                                        q * QCH : (q + 1) * QCH,
                                    ],
                                    start=(gi == 0 and t2 == 0),
                                    stop=(gi == NGROUPS - 1 and t2 == NDT - 1),
                                    perf_mode=DR,
                                )
                        gi += 1
                        if h == 1 and r == 1:
                            resync(f"rs{l}", a_sb[0:1, 0, 0:1])

                h_sb = hpool.tile([c_out, ROWS], BF16, tag="h", name=f"h{l}")
                for q in range(NQ):
                    if l == 0:
                        # H1 chunk = relu((Ahat@X)chunk @ W0 / s + b0)
                        p1 = ypool.tile(
                            [DIMS[0], QCH], F32, tag="p1", name=f"p1_{q}"
                        )
                        nc.vector.tensor_copy(out=p1[:], in_=acc[q][: DIMS[0], :])
                        ps2 = psml.tile(
                            [DIMS[1], QCH], F32, tag="psy", name=f"ps2_{q}"
                        )
                        nc.tensor.matmul(
                            ps2[:], lhsT=w_sb[0][:], rhs=p1[:],
                            start=True, stop=True,
                        )
                        nc.scalar.activation(
                            h_sb[:, q * QCH : (q + 1) * QCH],
                            ps2[:],
                            AF.Relu,
                            bias=b_sb[0][:],
                            scale=1.0 / (ASCALE * XSCALE),
                        )
                    else:
                        nc.scalar.activation(
                            h_sb[:, q * QCH : (q + 1) * QCH],
                            acc[q][:c_out, :],
                            AF.Relu,
                            bias=b_sb[l][:],
                            scale=1.0 / ASCALE,
                        )
                if l == 2:
                    break

                # ---- project local Y_{l+1} rows + two pipelined AllGathers
                c_next = DIMS[l + 2]
                stat = stat_pair(l + 1, c_next)
                for h in range(NHALF):
                    y_sb = ypool.tile(
                        [P, NU // 2, c_next], F8, tag="y", name=f"y{l}_{h}"
                    )
                    for u8 in range(NU // 2):
                        u = h * 8 + u8
                        ps = psml.tile(
                            [P, c_next], F32, tag="psy", name=f"psy{l}_{u}"
                        )
                        nc.tensor.matmul(
                            ps[:],
                            lhsT=h_sb[:, u * P : (u + 1) * P],
                            rhs=w_sb[l + 1][:],
                            start=True,
                            stop=True,
                        )
                        nc.vector.tensor_copy(out=y_sb[:, u8, :], in_=ps[:])
                    ag_in = dram.tile(
                        [P, NU // 2, c_next], F8, name=f"agin{l}_{h}"
                    )
                    ag_out = dram.tile(
                        [NCORES, P, NU // 2, c_next], F8, name=f"agout{l}_{h}",
                        addr_space="Shared",
                    )
                    nc.scalar.dma_start(ag_in[:], y_sb[:])
                    nc.gpsimd.collective_compute(
                        "AllGather",
                        OP.bypass,
                        replica_groups=rg,
                        ins=[ag_in[:].opt()],
                        outs=[ag_out[:].opt()],
                    )
                    nc.scalar.dma_start(
                        stat[h][:], ag_out[:].rearrange("r p u c -> p r u c")
                    )

            # ---- mean pool over all nodes ----
            gp = smal.tile([DIMS[3], 1], F32, name="gpart")
            nc.vector.tensor_reduce(
                gp[:], h_sb[:], axis=mybir.AxisListType.X, op=OP.add
            )
            ar_in = dram.tile([DIMS[3], 1], F32, name="arin")
            ar_out = dram.tile([DIMS[3], 1], F32, name="arout", addr_space="Shared")
            nc.scalar.dma_start(ar_in[:], gp[:])
            nc.gpsimd.collective_compute(
                "AllReduce",
                OP.add,
                replica_groups=rg,
                ins=[ar_in[:].opt()],
                outs=[ar_out[:].opt()],
            )
            g_sb = smal.tile([DIMS[3], 1], F32, name="gsb")
            nc.scalar.dma_start(g_sb[:], ar_out[:])
            nc.any.tensor_scalar_mul(g_sb[:], g_sb[:], 1.0 / N)

            # ---- MLP head: h1 = elu(g @ Wh1 + bh1) ----
            ps1 = psmlp.tile([32, 1], F32, tag="mlp", name="ps1")
            nc.tensor.matmul(ps1[:], lhsT=wh1_sb[:], rhs=g_sb[:], start=True, stop=True)
            # elu(x) = relu(x) + exp(min(x, 0)) - 1
            tmin = smal.tile([32, 1], F32, name="tmin")
            nc.vector.tensor_scalar(tmin[:], ps1[:], bh1_sb[:], 0.0, OP.add, OP.min)
            e1 = smal.tile([32, 1], F32, name="e1")
            nc.scalar.activation(e1[:], tmin[:], AF.Exp)
            r1 = smal.tile([32, 1], F32, name="r1")
            nc.scalar.activation(r1[:], ps1[:], AF.Relu, bias=bh1_sb[:])
            h1 = smal.tile([32, 1], F32, name="h1")
            nc.vector.tensor_tensor(h1[:], e1[:], r1[:], OP.add)
            nc.vector.tensor_scalar_add(h1[:], h1[:], -1.0)

            # ---- logits = h1 @ Wh2 + bh2; probs = softmax(logits) ----
            ps2m = psmlp.tile([2, 1], F32, tag="mlp", name="ps2m")
            nc.tensor.matmul(ps2m[:], lhsT=wh2_sb[:], rhs=h1[:], start=True, stop=True)
            logit_sb = smal.tile([2, 1], F32, name="logitsb")
            nc.vector.tensor_scalar(logit_sb[:], ps2m[:], bh2_sb[:], None, OP.add)
            nc.scalar.dma_start(logits_o.ap(), logit_sb[:])

            e2 = smal.tile([2, 1], F32, name="e2")
            nc.scalar.activation(e2[:], ps2m[:], AF.Exp, bias=bh2_sb[:])
            ones21 = smal.tile([2, 1], F32, name="ones21")
            nc.any.memset(ones21[:], 1.0)
            ones12 = smal.tile([1, 2], F32, name="ones12")
            nc.any.memset(ones12[:], 1.0)
            ps3 = psmlp.tile([1, 1], F32, tag="mlp", name="ps3")
            nc.tensor.matmul(ps3[:], lhsT=e2[:], rhs=ones21[:], start=True, stop=True)
            rsc = smal.tile([1, 1], F32, name="rsc")
            nc.vector.reciprocal(rsc[:], ps3[:])
            ps4 = psmlp.tile([2, 1], F32, tag="mlp", name="ps4")
            nc.tensor.matmul(ps4[:], lhsT=ones12[:], rhs=rsc[:], start=True, stop=True)
            probs_sb = smal.tile([2, 1], F32, name="probssb")
            nc.vector.tensor_tensor(probs_sb[:], e2[:], ps4[:], OP.mult)
            nc.scalar.dma_start(probs_o.ap(), probs_sb[:])

    nc.finalize()
    return nc


def _install_ntff_hook():
    """Register the axon NTFF profiling hook if the container's antenv stub
    lacks it (bass_utils imports antenv.axon_hooks when trace=True)."""
    import sys
    import types

    try:
        import antenv.axon_hooks  # noqa: F401
        return
    except ImportError:
        pass
    mod = types.ModuleType("antenv.axon_hooks")
    _h = [None]
    mod.set_axon_ntff_profile_hook = lambda h: _h.__setitem__(0, h)
    mod.get_axon_ntff_profile_hook = lambda: _h[0]
    sys.modules["antenv.axon_hooks"] = mod
    import antenv

    antenv.axon_hooks = mod
    try:
        from trn_agent_boot import trn_boot

        hook = trn_boot._ntff_profile_via_ctypes("/opt/axon/libaxon_pjrt.so")
        if hook is not None:
            mod.set_axon_ntff_profile_hook(hook)
    except Exception:
        pass


def _get_nc():
    global _nc_cache
    if _nc_cache is None:
        _nc_cache = _build_nc()
    return _nc_cache


_last_results = None


def kernel(
    node_feat,
    adj_matrix,
    W0,
    b0,
    W1,
    b1,
    W2,
    b2,
    Wh1,
    bh1,
    Wh2,
    bh2,
):
    global _last_results
    import os

    node_feat = np.ascontiguousarray(np.asarray(node_feat, dtype=np.float32))
    adj = np.asarray(adj_matrix, dtype=np.float32)

    # ---- host-side sharding / preprocessing ----
    deg = adj.sum(axis=1, dtype=np.float32) + 1.0
    dinv = (1.0 / np.sqrt(deg)).astype(np.float32)

    fp8 = ml_dtypes.float8_e4m3
    bf16 = ml_dtypes.bfloat16
    f32c = lambda a, shape=None: np.ascontiguousarray(
        np.asarray(a, dtype=np.float32).reshape(shape)
        if shape is not None
        else np.asarray(a, dtype=np.float32)
    )

    # X scaled to fp8, tiled [p, h, r, u8, c]: node j = r*2048+h*1024+u8*128+p
    x8 = (node_feat * np.float32(XSCALE)).astype(fp8)
    x8 = np.ascontiguousarray(
        x8.reshape(NCORES, NHALF, NU // 2, P, DIMS[0]).transpose(3, 1, 0, 2, 4)
    )

    common = {
        "x8": x8,
        "w0": f32c(W0),
        "b0": f32c(b0, (-1, 1)),
        "w1": np.ascontiguousarray(np.asarray(W1, np.float32)).astype(bf16),
        "b1": f32c(b1, (-1, 1)),
        "w2": np.ascontiguousarray(np.asarray(W2, np.float32)).astype(bf16),
        "b2": f32c(b2, (-1, 1)),
        "wh1": f32c(Wh1),
        "bh1": f32c(bh1, (-1, 1)),
        "wh2": f32c(Wh2),
        "bh2": f32c(bh2, (-1, 1)),
    }

    in_maps = []
    idx = np.arange(ROWS)
    sdinv = dinv * np.float32(ASCALE)
    for k in range(NCORES):
        sl = slice(k * ROWS, (k + 1) * ROWS)
        # rows of ASCALE*Ahat for this core's output nodes
        blk = adj[sl, :] * sdinv[sl, None]
        blk *= dinv[None, :]
        blk[idx, k * ROWS + idx] = sdinv[sl] * dinv[sl]  # + I self loops
        a_k = blk.T.astype(fp8)  # [N, ROWS] = scaled Ahat.T cols
        # pre-tile to device layout [h, r, p, t, i]:
        # row j = r*2048 + h*1024 + t*128 + p
        a_k = np.ascontiguousarray(
            a_k.reshape(NCORES, NHALF, NSTRIPE, P, ROWS).transpose(1, 0, 3, 2, 4)
        )
        m = {"a_t": a_k}
        m.update(common)
        in_maps.append(m)

    from concourse import bass_utils

    nc = _get_nc()
    trace = bool(int(os.environ.get("GCN_TRACE", "0")))
    if trace:
        _install_ntff_hook()
    res = bass_utils.run_bass_kernel_spmd(
        nc, in_maps, core_ids=list(range(NCORES)), trace=trace
    )
    _last_results = res

    out0 = res.results[0]
    logits = np.asarray(out0["logits"], dtype=np.float32).reshape(2)
    probs = np.asarray(out0["probs"], dtype=np.float32).reshape(2)
    return (logits, probs)
